# revision 47
# baseline (speedup 1.0000x reference)
"""LDPC belief-propagation kernel for Trainium2 (8 NeuronCores, data-parallel).

Math (per batch row, H fixed [3,7], 12 edges):
  t_e   = tanh(m_e / 2)                       (signed!)
  u_e   = prod_{e' in check c, e' != e} t_e'  (signed leave-one-out product)
  c2v_e = 2 atanh(u_e) = ln(1+u) - ln(1-u)    (signed, sign handled for free)
  new_llr_v = llr_v + sum_{c contains v} c2v_{c,v}
  m'_e  = new_llr_v - c2v_e
Only Tanh/Ln tables; the sign pipeline of the classic phi/phi formulation
disappears because the tanh products carry signs natively.

Edge layout is role-major per 12-slot group: [s0 s1 s2 | a0 a1 a2 | b0 b1 b2 |
d0 d1 d2] where s_c is the check's degree-1-variable edge (v0,v1,v3 - their
messages never change), d_c is v6's edge in check c, and (a, b) =
((v2c0, v2c1, v4c2), (v4c0, v5c1, v5c2)).  This makes every structural op a
single strided instruction:
  Q[k]   = T[k] * T[k+6]         (k=0..5: pair products (s*b, a*d) per check)
  U[3..8]  = T[(9,10,11,0,1,2)] * Q[0..5]   (loo for roles a, b)
  U[9..11] = T[3..5] * Q[0..2]              (loo for role d)
  M'[deg2 six edges] = LB6 + CV[partner]    (partner = pair-swap view)
  M'[d]  = (c2v_d-sum + llr_6) - CV[d]      (v6 leave-one-out via total sum)
Batch is split into 3 width-tuned chunks (84/88/84 columns per partition)
whose iterations run staggered, so ACT (Tanh + the two Ln ops), DVE
(products, c2v subtract, message updates) and Pool (d-role products, v6
pair-sums) stay concurrently busy; iteration 0 reads tanh(llr) directly so
the edge-slot scatter stays off the pipeline-fill critical path, and
setup copies / output DMAs are placed by schedule hints to keep the fill
and drain edges tight.  Sync info is reduced to walrus's one-wait-slot
limit by a vector-clock pass (_strip_syncs).
"""

import numpy as np

_CACHE = {}

NCORES = 8
P = 128      # partitions
CHUNKS = 3   # batch sub-chunks per core (pipeline depth)

# guard so ln(1 -+ 0.99999988*u) stays finite (>= ~1.2e-7) even at u = -+1
LNSCALE = 0.99999988

# (start, half-period, partA->partB gap) ns hints for the list scheduler
_SCHED = (0, 0, 0)

# schedule hints deferring chunk setup copies (ns)
_CPY = (0, 9800, 12300)

# manual chunk widths (must sum to Bc//P//1); None = near-even
_WS = (84, 88, 84)


def _build(Bc, iters):
    import contextlib

    import concourse.bass as bass
    import concourse.tile as tile
    from concourse import mybir
    from concourse.alu_op_type import AluOpType as Op

    F = mybir.ActivationFunctionType
    Wtot = Bc // P
    if _WS is not None and sum(_WS) == Wtot:
        Ws = list(_WS)
    else:
        base, rem = divmod(Wtot, CHUNKS)
        Ws = [base + (1 if i == 1 else 0) for i in range(CHUNKS)] \
            if rem == 1 else \
            [base + (1 if i >= CHUNKS - rem else 0) for i in range(CHUNKS)]
    f32 = mybir.dt.float32

    f16 = mybir.dt.float16
    nc = bass.Bass("TRN2", target_bir_lowering=False, debug=False,
                   num_devices=1)
    llr_d = nc.dram_tensor("llr", [Bc, 7], f32, kind="ExternalInput")
    out_d = nc.dram_tensor("out", [Bc, 7], f32, kind="ExternalOutput")

    def sub(t, off, dims):
        a = t[:] if callable(getattr(t, "__getitem__", None)) else t
        return bass.AP(tensor=a.tensor, offset=a.offset + off,
                       ap=[list(a.ap[0])] + [list(d) for d in dims])

    with tile.TileContext(nc) as tc:
        ctx = contextlib.ExitStack()
        with ctx:
            keep = ctx.enter_context(tc.tile_pool(name="keep", bufs=1))
            work = ctx.enter_context(tc.tile_pool(name="work", bufs=2))

            def K(name, c, k, dt=f32):
                return keep.tile([P, Ws[c] * k], dt, tag=name, name=name)

            # per-chunk persistent state
            LLs = [K(f"LL{c}", c, 7) for c in range(CHUNKS)]    # llr, natural v order
            LBs = [K(f"LB{c}", c, 6, f16) for c in range(CHUNKS)]   # llr bcast, deg2 edges
            L6s = [K(f"L6{c}", c, 3, f16) for c in range(CHUNKS)]   # llr6 bcast, v6 edges
            Ts  = [K(f"T{c}", c, 12) for c in range(CHUNKS)]    # tanh(m/2) per edge
            Ms  = [K(f"M{c}", c, 9, f16) for c in range(CHUNKS)]    # dyn messages
            NLs = [K(f"NL{c}", c, 7) for c in range(CHUNKS)]    # output llr

            act = nc.scalar.activation
            vec = nc.vector
            gps = nc.gpsimd

            def dram_view(t, c, w0, nw):
                # [P, nw*7] window of chunk c: rows base_c + p*Ws[c] + w
                a = t.ap()
                off = (P * sum(Ws[:c]) + w0) * 7
                return bass.AP(tensor=a.tensor, offset=a.offset + off,
                               ap=[[Ws[c] * 7, P], [1, nw * 7]])

            for c in range(CHUNKS):
                eng = nc.sync if c == 0 else nc.gpsimd
                eng.dma_start(out=LLs[c][:], in_=dram_view(llr_d, c, 0, Ws[c]))

            cur = {"W": Ws[0]}

            def v7(t, off, *dims):
                return sub(t, off, [[7, cur["W"]]] + [list(d) for d in dims])

            def v12(t, off, *dims):
                return sub(t, off, [[12, cur["W"]]] + [list(d) for d in dims])

            def v9(t, off, *dims):
                return sub(t, off, [[9, cur["W"]]] + [list(d) for d in dims])

            def v6(t, off, *dims):
                return sub(t, off, [[6, cur["W"]]] + [list(d) for d in dims])

            def v3(t, off, *dims):
                return sub(t, off, [[3, cur["W"]]] + [list(d) for d in dims])

            state = [{} for _ in range(CHUNKS)]

            def partA(c, it):
                """tanh + products: T, Q, U."""
                LL, LB, T, M = LLs[c], LBs[c], Ts[c], Ms[c]
                last = (it == iters - 1)
                W = Ws[c]
                cur["W"] = W
                Q = work.tile([P, W * 6], f32, tag=f"Q{c}", name="Q")
                U = work.tile([P, W * 12], f32, tag=f"U{c}", name="U")
                state[c] = {"Q": Q, "U": U}

                if it == 0:
                    # t = tanh(llr/2) once; iteration-0 products read TL
                    # directly so the T scatter stays off the critical path
                    TL = work.tile([P, W * 7], f32, tag=f"TL{c}", name="TL")
                    act(TL[:], LL[:], F.Tanh, scale=0.5)
                    vec.tensor_tensor(v6(Q, 0, [1, 2]), v7(TL, 0, [1, 2]),
                                      v7(TL, 4, [1, 2]), Op.mult)
                    vec.tensor_tensor(v6(Q, 2, [1, 1]), v7(TL, 3, [1, 1]),
                                      v7(TL, 5, [1, 1]), Op.mult)
                    gps.tensor_tensor(v6(Q, 3, [1, 2]), v7(TL, 2, [0, 2]),
                                      v7(TL, 6, [0, 2]), Op.mult)
                    gps.tensor_tensor(v6(Q, 5, [1, 1]), v7(TL, 4, [1, 1]),
                                      v7(TL, 6, [1, 1]), Op.mult)
                    vec.tensor_tensor(v12(U, 3, [1, 3]), v7(TL, 6, [0, 3]),
                                      v6(Q, 0, [1, 3]), Op.mult)
                    vec.tensor_tensor(v12(U, 6, [1, 2]), v7(TL, 0, [1, 2]),
                                      v6(Q, 3, [1, 2]), Op.mult)
                    vec.tensor_tensor(v12(U, 8, [1, 1]), v7(TL, 3, [1, 1]),
                                      v6(Q, 5, [1, 1]), Op.mult)
                    gps.tensor_tensor(v12(U, 9, [1, 2]), v7(TL, 2, [0, 2]),
                                      v6(Q, 0, [1, 2]), Op.mult)
                    gps.tensor_tensor(v12(U, 11, [1, 1]), v7(TL, 4, [1, 1]),
                                      v6(Q, 2, [1, 1]), Op.mult)
                    if last:  # iters == 1
                        vec.tensor_tensor(v12(U, 0, [1, 2]), v7(TL, 4, [1, 2]),
                                          v6(Q, 3, [1, 2]), Op.mult)
                        vec.tensor_tensor(v12(U, 2, [1, 1]), v7(TL, 5, [1, 1]),
                                          v6(Q, 5, [1, 1]), Op.mult)
                    # scatter t to role-major slots for later iterations,
                    # off the critical path (only statics strictly needed
                    # before iteration 1's products)
                    vec.tensor_copy(v12(T, 0, [1, 2]), v7(TL, 0, [1, 2]))
                    vec.tensor_copy(v12(T, 2, [1, 1]), v7(TL, 3, [1, 1]))
                    with tc.tile_wait_until(_CPY[c] / 1e6, enable=_CPY[c] > 0):
                        gps.tensor_copy(v6(LB, 0, [1, 2]), v7(LL, 2, [0, 2]))
                        gps.tensor_copy(v6(LB, 2, [1, 4]),
                                        v7(LL, 4, [1, 2], [0, 2]))
                        gps.tensor_copy(v3(L6s[c], 0, [1, 3]),
                                        v7(LL, 6, [0, 3]))
                else:
                    act(v12(T, 3, [1, 9]), M[:], F.Tanh, scale=0.5)
                    # pair products and signed leave-one-out products
                    vec.tensor_tensor(Q[:], v12(T, 0, [1, 6]),
                                      v12(T, 6, [1, 6]), Op.mult)
                    vec.tensor_tensor(v12(U, 3, [1, 6]),
                                      v12(T, 9, [-9, 2], [1, 3]),
                                      v6(Q, 0, [1, 6]), Op.mult)
                    gps.tensor_tensor(v12(U, 9, [1, 3]), v12(T, 3, [1, 3]),
                                      v6(Q, 0, [1, 3]), Op.mult)
                    if last:
                        vec.tensor_tensor(v12(U, 0, [1, 3]), v12(T, 6, [1, 3]),
                                          v6(Q, 3, [1, 3]), Op.mult)

            def partB(c, it):
                """c2v + message/new-llr update."""
                LL, LB, M, NL = LLs[c], LBs[c], Ms[c], NLs[c]
                last = (it == iters - 1)
                W = Ws[c]
                cur["W"] = W
                U = state[c]["U"]
                LP = work.tile([P, W * 12], f16, tag=f"LP{c}", name="LP")
                LM = work.tile([P, W * 12], f16, tag=f"LM{c}", name="LM")
                CV = work.tile([P, W * 12], f16, tag=f"CV{c}", name="CV")

                off, n = (0, 9) if last else (3, 6)
                # c2v = ln(1+u) - ln(1-u), guarded away from ln(0)
                act(v12(LP, off, [1, n + 3]), v12(U, off, [1, n + 3]), F.Ln,
                    bias=1.0, scale=LNSCALE)
                act(v12(LM, off, [1, n + 3]), v12(U, off, [1, n + 3]), F.Ln,
                    bias=1.0, scale=-LNSCALE)
                vec.tensor_tensor(v12(CV, off, [1, n + 3]),
                                  v12(LP, off, [1, n + 3]),
                                  v12(LM, off, [1, n + 3]), Op.subtract)

                # v6 leave-one-out sums of the d-role c2vs, depth 2:
                # X[0]=c10+c11  X[1]=c9+c11  X[2]=c9+c10
                X = work.tile([P, W * 3], f16, tag=f"X{c}", name="X")
                vec.tensor_tensor(v3(X, 0, [1, 2]), v12(CV, 10, [-1, 2]),
                                  v12(CV, 11, [0, 2]), Op.add)
                gps.tensor_tensor(v3(X, 2, [1, 1]), v12(CV, 9, [1, 1]),
                                  v12(CV, 10, [1, 1]), Op.add)

                if not last:
                    # m' for the six deg-2 edges: llr + partner c2v
                    vec.tensor_tensor(v9(M, 0, [1, 6]), v6(LB, 0, [1, 6]),
                                      v12(CV, 4, [2, 3], [-1, 2]), Op.add)
                    # m' for v6 edges: llr6 + sum of the other two c2v_d
                    vec.tensor_tensor(v9(M, 6, [1, 3]), v3(L6s[c], 0, [1, 3]),
                                      v3(X, 0, [1, 3]), Op.add)
                else:
                    # new_llr in natural variable order
                    SP = work.tile([P, W * 3], f32, tag=f"SP{c}", name="SP")
                    gps.tensor_tensor(v7(NL, 0, [1, 2]), v7(LL, 0, [1, 2]),
                                      v12(CV, 0, [1, 2]), Op.add)
                    gps.tensor_tensor(v7(NL, 3, [1, 1]), v7(LL, 3, [1, 1]),
                                      v12(CV, 2, [1, 1]), Op.add)
                    vec.tensor_tensor(SP[:], v12(CV, 3, [2, 3]),
                                      v12(CV, 4, [2, 3]), Op.add)
                    vec.tensor_tensor(v7(NL, 2, [1, 1]), v7(LL, 2, [1, 1]),
                                      v3(SP, 0, [1, 1]), Op.add)
                    vec.tensor_tensor(v7(NL, 4, [1, 2]), v7(LL, 4, [1, 2]),
                                      v3(SP, 1, [1, 2]), Op.add)
                    S1 = work.tile([P, W], f32, tag=f"S1{c}", name="S1")
                    vec.tensor_tensor(S1[:], v3(X, 2, [1, 1]),
                                      v12(CV, 11, [1, 1]), Op.add)
                    vec.tensor_tensor(v7(NL, 6, [1, 1]), S1[:],
                                      v7(LL, 6, [1, 1]), Op.add)
                    wl = W // 2
                    wh = W - wl
                    lo = bass.AP(tensor=NL[:].tensor, offset=NL[:].offset,
                                 ap=[list(NL[:].ap[0])] + [[7, wl], [1, 7]])
                    hi = bass.AP(tensor=NL[:].tensor,
                                 offset=NL[:].offset + wl * 7,
                                 ap=[list(NL[:].ap[0])] + [[7, wh], [1, 7]])
                    e0, e1 = ((nc.sync, nc.sync) if c < CHUNKS - 1
                              else (nc.sync, nc.gpsimd))
                    e0.dma_start(out=dram_view(out_d, c, 0, wl), in_=lo)
                    e1.dma_start(out=dram_view(out_d, c, wl, wh), in_=hi)

            # software-pipelined schedule: chunk 1 runs half an iteration
            # behind chunk 0 so each chunk's ACT phase (Tanh / Ln Ln) overlaps
            # the other chunk's vector phase (products / updates).  The
            # wait-until timestamps steer the Tile list scheduler into that
            # stagger; they are lower bounds only, data deps still rule.
            S0, HALF, GAP = _SCHED
            for it in range(iters):
                for c in range(CHUNKS):
                    tA = S0 + (CHUNKS * it + c) * HALF
                    with tc.tile_wait_until(tA / 1e6, enable=tA > 0):
                        partA(c, it)
                    with tc.tile_wait_until((tA + GAP) / 1e6):
                        partB(c, it)

    _strip_syncs(nc)
    return nc


def _strip_syncs(nc):
    """walrus on this stack supports a single sync-wait slot per instruction.
    Reduce each instruction's wait list via a vector-clock pass: walking the
    scheduled program order, every engine accumulates knowledge of semaphore
    values - from its own queue position, from waits it has already performed,
    and transitively from the producer's knowledge snapshot at the awaited
    update.  A wait already implied by that knowledge is dropped.  Kernel-tail
    drains keep only their DMA wait (the per-engine drain + EVSEM butterfly
    that follows enforces engine completion)."""
    import bass_rust

    eng_sem = {"EngineType.DVE": "DVE_", "EngineType.Pool": "Pool_",
               "EngineType.Activation": "Activation_", "EngineType.PE": "PE_",
               "EngineType.SP": "SP_"}
    know = {e: {} for e in eng_sem}          # engine -> {sem: value}
    sem_hist = {}                            # sem -> list of (cum_value, snapshot)
    sem_cum = {}                             # sem -> cumulative inc so far

    # Sems that are ever decremented (barrier gather sems) are not monotone;
    # leave their waits untouched and keep them out of the knowledge model.
    nonmono = set()
    for b in nc.m.functions[0].blocks:
        for inst in b.instructions:
            si = inst.sync_info
            if si is not None:
                for u in si.on_update:
                    if u.update_mode != "sem-inc":
                        nonmono.add(u.ant_name)

    def implied(k, sem, val):
        return k.get(sem, 0) >= val

    def learn(k, sem, val):
        if k.get(sem, 0) < val:
            k[sem] = val
        # transitively absorb the producer's snapshot at this update
        hist = sem_hist.get(sem)
        if hist:
            import bisect
            i = bisect.bisect_left([h[0] for h in hist], val)
            if i < len(hist):
                for s2, v2 in hist[i][1].items():
                    if k.get(s2, 0) < v2:
                        k[s2] = v2

    from concourse import mybir

    for b in nc.m.functions[0].blocks:
        new_instructions = []
        for inst in b.instructions:
            si = inst.sync_info
            eng = str(inst.engine)
            k = know.setdefault(eng, {})
            if si is not None:
                waits = list(si.on_wait)
                if type(inst).__name__ == "InstDrain" and len(waits) > 1:
                    dma = [w for w in waits if "DMA" in w.ant_name]
                    keep_w = dma[-1:] if dma else waits[:1]
                    for w in waits:
                        learn(k, w.ant_name, w.wait_value)
                else:
                    merged = {}
                    for w in waits:
                        if w.ant_name in nonmono:
                            merged[id(w)] = w
                        elif w.ant_name not in merged or \
                                merged[w.ant_name].wait_value < w.wait_value:
                            merged[w.ant_name] = w
                    keep_w = []
                    for w in merged.values():
                        if w.ant_name in nonmono:
                            keep_w.append(w)
                            continue
                        if not implied(k, w.ant_name, w.wait_value):
                            keep_w.append(w)
                        learn(k, w.ant_name, w.wait_value)
                    # walrus has one wait slot per instruction: hoist extra
                    # waits onto injected no-ops on the same engine
                    while len(keep_w) > 1:
                        w = keep_w.pop(0)
                        nop = mybir.InstNoOp(
                            name=f"{inst.name}_w{len(keep_w)}",
                            engine=inst.engine, ins=[], outs=[],
                            sync_info=bass_rust.SyncInfo(
                                on_wait=[w], on_update=[]))
                        new_instructions.append(nop)
                if len(keep_w) != len(waits):
                    inst.sync_info = bass_rust.SyncInfo(
                        on_wait=keep_w, on_update=list(si.on_update))
                    si = inst.sync_info
                for u in si.on_update:
                    if u.update_mode == "sem-inc" and u.ant_name not in nonmono:
                        name = u.ant_name
                        cum = sem_cum.get(name, 0) + u.update_value
                        sem_cum[name] = cum
                        # own-engine sems are implicitly ordered for later
                        # instructions on the same queue
                        pref = eng_sem.get(eng)
                        if pref and name.startswith(pref):
                            k[name] = max(k.get(name, 0), cum)
                        sem_hist.setdefault(name, []).append((cum, dict(k)))
            new_instructions.append(inst)
        if len(new_instructions) != len(b.instructions):
            b.instructions = new_instructions


def kernel(llr, max_iters):
    llr = np.ascontiguousarray(np.asarray(llr), dtype=np.float32)
    iters = int(np.asarray(max_iters))
    B = llr.shape[0]
    if iters <= 0:
        return llr.reshape(B, 1, 7).copy()

    from concourse.bass_utils import run_bass_kernel_spmd

    Bc = B // NCORES
    key = (Bc, iters)
    if key not in _CACHE:
        _CACHE[key] = _build(Bc, iters)
    nc = _CACHE[key]

    flat = llr.reshape(B, 7)
    in_maps = [{"llr": flat[i * Bc:(i + 1) * Bc]} for i in range(NCORES)]
    res = run_bass_kernel_spmd(nc, in_maps, core_ids=list(range(NCORES)))
    out = np.concatenate([np.asarray(r["out"]) for r in res.results], axis=0)
    return out.reshape(B, 1, 7)


# revision 49
# speedup vs baseline: 1.0059x; 1.0059x over previous
"""LDPC belief-propagation kernel for Trainium2 (8 NeuronCores, data-parallel).

Math (per batch row, H fixed [3,7], 12 edges):
  t_e   = tanh(m_e / 2)                       (signed!)
  u_e   = prod_{e' in check c, e' != e} t_e'  (signed leave-one-out product)
  c2v_e = 2 atanh(u_e) = ln(1+u) - ln(1-u)    (signed, sign handled for free)
  new_llr_v = llr_v + sum_{c contains v} c2v_{c,v}
  m'_e  = new_llr_v - c2v_e
Only Tanh/Ln tables; the sign pipeline of the classic phi/phi formulation
disappears because the tanh products carry signs natively.

Edge layout is role-major per 12-slot group: [s0 s1 s2 | a0 a1 a2 | b0 b1 b2 |
d0 d1 d2] where s_c is the check's degree-1-variable edge (v0,v1,v3 - their
messages never change), d_c is v6's edge in check c, and (a, b) =
((v2c0, v2c1, v4c2), (v4c0, v5c1, v5c2)).  This makes every structural op a
single strided instruction:
  Q[k]   = T[k] * T[k+6]         (k=0..5: pair products (s*b, a*d) per check)
  U[3..8]  = T[(9,10,11,0,1,2)] * Q[0..5]   (loo for roles a, b)
  U[9..11] = T[3..5] * Q[0..2]              (loo for role d)
  M'[deg2 six edges] = LB6 + CV[partner]    (partner = pair-swap view)
  M'[d]  = (c2v_d-sum + llr_6) - CV[d]      (v6 leave-one-out via total sum)
Batch is split into 3 width-tuned chunks (84/88/84 columns per partition)
whose iterations run staggered, so ACT (Tanh + the two Ln ops), DVE
(products, c2v subtract, message updates) and Pool (d-role products, v6
pair-sums) stay concurrently busy; iteration 0 reads tanh(llr) directly so
the edge-slot scatter stays off the pipeline-fill critical path, and
setup copies / output DMAs are placed by schedule hints to keep the fill
and drain edges tight.  Sync info is reduced to walrus's one-wait-slot
limit by a vector-clock pass (_strip_syncs).
"""

import numpy as np

_CACHE = {}

NCORES = 8
P = 128      # partitions
CHUNKS = 3   # batch sub-chunks per core (pipeline depth)

# guard so ln(1 -+ 0.99999988*u) stays finite (>= ~1.2e-7) even at u = -+1
LNSCALE = 0.99999988

# (start, half-period, partA->partB gap) ns hints for the list scheduler
_SCHED = (0, 0, 0)

# schedule hints deferring chunk setup copies (ns)
_CPY = (0, 9800, 12300)

# manual chunk widths (must sum to Bc//P//1); None = near-even
_WS = (84, 88, 84)


def _build(Bc, iters):
    import contextlib

    import concourse.bass as bass
    import concourse.tile as tile
    from concourse import mybir
    from concourse.alu_op_type import AluOpType as Op

    F = mybir.ActivationFunctionType
    Wtot = Bc // P
    if _WS is not None and sum(_WS) == Wtot:
        Ws = list(_WS)
    else:
        base, rem = divmod(Wtot, CHUNKS)
        Ws = [base + (1 if i == 1 else 0) for i in range(CHUNKS)] \
            if rem == 1 else \
            [base + (1 if i >= CHUNKS - rem else 0) for i in range(CHUNKS)]
    f32 = mybir.dt.float32

    f16 = mybir.dt.float16
    nc = bass.Bass("TRN2", target_bir_lowering=False, debug=False,
                   num_devices=1)
    llr_d = nc.dram_tensor("llr", [Bc, 7], f32, kind="ExternalInput")
    out_d = nc.dram_tensor("out", [Bc, 7], f32, kind="ExternalOutput")

    def sub(t, off, dims):
        a = t[:] if callable(getattr(t, "__getitem__", None)) else t
        return bass.AP(tensor=a.tensor, offset=a.offset + off,
                       ap=[list(a.ap[0])] + [list(d) for d in dims])

    with tile.TileContext(nc) as tc:
        ctx = contextlib.ExitStack()
        with ctx:
            keep = ctx.enter_context(tc.tile_pool(name="keep", bufs=1))
            work = ctx.enter_context(tc.tile_pool(name="work", bufs=2))

            def K(name, c, k, dt=f32):
                return keep.tile([P, Ws[c] * k], dt, tag=name, name=name)

            # per-chunk persistent state
            LLs = [K(f"LL{c}", c, 7) for c in range(CHUNKS)]    # llr, natural v order
            LBs = [K(f"LB{c}", c, 6, f16) for c in range(CHUNKS)]   # llr bcast, deg2 edges
            L6s = [K(f"L6{c}", c, 3, f16) for c in range(CHUNKS)]   # llr6 bcast, v6 edges
            Ts  = [K(f"T{c}", c, 12) for c in range(CHUNKS)]    # tanh(m/2) per edge
            Ms  = [K(f"M{c}", c, 9, f16) for c in range(CHUNKS)]    # dyn messages
            NLs = [K(f"NL{c}", c, 7) for c in range(CHUNKS)]    # output llr

            act = nc.scalar.activation
            vec = nc.vector
            gps = nc.gpsimd

            def dram_view(t, c, w0, nw):
                # [P, nw*7] window of chunk c: rows base_c + p*Ws[c] + w
                a = t.ap()
                off = (P * sum(Ws[:c]) + w0) * 7
                return bass.AP(tensor=a.tensor, offset=a.offset + off,
                               ap=[[Ws[c] * 7, P], [1, nw * 7]])

            for c in range(CHUNKS):
                eng = nc.sync if c == 0 else nc.gpsimd
                eng.dma_start(out=LLs[c][:], in_=dram_view(llr_d, c, 0, Ws[c]))

            cur = {"W": Ws[0]}

            def v7(t, off, *dims):
                return sub(t, off, [[7, cur["W"]]] + [list(d) for d in dims])

            def v12(t, off, *dims):
                return sub(t, off, [[12, cur["W"]]] + [list(d) for d in dims])

            def v9(t, off, *dims):
                return sub(t, off, [[9, cur["W"]]] + [list(d) for d in dims])

            def v6(t, off, *dims):
                return sub(t, off, [[6, cur["W"]]] + [list(d) for d in dims])

            def v3(t, off, *dims):
                return sub(t, off, [[3, cur["W"]]] + [list(d) for d in dims])

            state = [{} for _ in range(CHUNKS)]

            def partA(c, it):
                """tanh + products: T, Q, U."""
                LL, LB, T, M = LLs[c], LBs[c], Ts[c], Ms[c]
                last = (it == iters - 1)
                W = Ws[c]
                cur["W"] = W
                Q = work.tile([P, W * 6], f32, tag=f"Q{c}", name="Q")
                U = work.tile([P, W * 12], f32, tag=f"U{c}", name="U")
                state[c] = {"Q": Q, "U": U}

                if it == 0:
                    # t = tanh(llr/2) once; iteration-0 products read TL
                    # directly so the T scatter stays off the critical path
                    TL = work.tile([P, W * 7], f32, tag=f"TL{c}", name="TL")
                    act(TL[:], LL[:], F.Tanh, scale=0.5)
                    vec.tensor_tensor(v6(Q, 0, [1, 2]), v7(TL, 0, [1, 2]),
                                      v7(TL, 4, [1, 2]), Op.mult)
                    vec.tensor_tensor(v6(Q, 2, [1, 1]), v7(TL, 3, [1, 1]),
                                      v7(TL, 5, [1, 1]), Op.mult)
                    gps.tensor_tensor(v6(Q, 3, [1, 2]), v7(TL, 2, [0, 2]),
                                      v7(TL, 6, [0, 2]), Op.mult)
                    gps.tensor_tensor(v6(Q, 5, [1, 1]), v7(TL, 4, [1, 1]),
                                      v7(TL, 6, [1, 1]), Op.mult)
                    vec.tensor_tensor(v12(U, 3, [1, 3]), v7(TL, 6, [0, 3]),
                                      v6(Q, 0, [1, 3]), Op.mult)
                    vec.tensor_tensor(v12(U, 6, [1, 2]), v7(TL, 0, [1, 2]),
                                      v6(Q, 3, [1, 2]), Op.mult)
                    vec.tensor_tensor(v12(U, 8, [1, 1]), v7(TL, 3, [1, 1]),
                                      v6(Q, 5, [1, 1]), Op.mult)
                    gps.tensor_tensor(v12(U, 9, [1, 2]), v7(TL, 2, [0, 2]),
                                      v6(Q, 0, [1, 2]), Op.mult)
                    gps.tensor_tensor(v12(U, 11, [1, 1]), v7(TL, 4, [1, 1]),
                                      v6(Q, 2, [1, 1]), Op.mult)
                    if last:  # iters == 1
                        vec.tensor_tensor(v12(U, 0, [1, 2]), v7(TL, 4, [1, 2]),
                                          v6(Q, 3, [1, 2]), Op.mult)
                        vec.tensor_tensor(v12(U, 2, [1, 1]), v7(TL, 5, [1, 1]),
                                          v6(Q, 5, [1, 1]), Op.mult)
                    # scatter t to role-major slots for later iterations,
                    # off the critical path (only statics strictly needed
                    # before iteration 1's products)
                    vec.tensor_copy(v12(T, 0, [1, 2]), v7(TL, 0, [1, 2]))
                    vec.tensor_copy(v12(T, 2, [1, 1]), v7(TL, 3, [1, 1]))
                    with tc.tile_wait_until(_CPY[c] / 1e6, enable=_CPY[c] > 0):
                        gps.tensor_copy(v6(LB, 0, [1, 2]), v7(LL, 2, [0, 2]))
                        gps.tensor_copy(v6(LB, 2, [1, 4]),
                                        v7(LL, 4, [1, 2], [0, 2]))
                        gps.tensor_copy(v3(L6s[c], 0, [1, 3]),
                                        v7(LL, 6, [0, 3]))
                else:
                    act(v12(T, 3, [1, 9]), M[:], F.Tanh, scale=0.5)
                    # pair products and signed leave-one-out products
                    vec.tensor_tensor(Q[:], v12(T, 0, [1, 6]),
                                      v12(T, 6, [1, 6]), Op.mult)
                    vec.tensor_tensor(v12(U, 3, [1, 6]),
                                      v12(T, 9, [-9, 2], [1, 3]),
                                      v6(Q, 0, [1, 6]), Op.mult)
                    gps.tensor_tensor(v12(U, 9, [1, 3]), v12(T, 3, [1, 3]),
                                      v6(Q, 0, [1, 3]), Op.mult)
                    if last:
                        vec.tensor_tensor(v12(U, 0, [1, 3]), v12(T, 6, [1, 3]),
                                          v6(Q, 3, [1, 3]), Op.mult)

            def partB(c, it):
                """c2v + message/new-llr update."""
                LL, LB, M, NL = LLs[c], LBs[c], Ms[c], NLs[c]
                last = (it == iters - 1)
                W = Ws[c]
                cur["W"] = W
                U = state[c]["U"]
                LP = work.tile([P, W * 12], f16, tag=f"LP{c}", name="LP")
                LM = work.tile([P, W * 12], f16, tag=f"LM{c}", name="LM")
                CV = work.tile([P, W * 12], f16, tag=f"CV{c}", name="CV")

                off, n = (0, 9) if last else (3, 6)
                # c2v = ln(1+u) - ln(1-u), guarded away from ln(0)
                act(v12(LP, off, [1, n + 3]), v12(U, off, [1, n + 3]), F.Ln,
                    bias=1.0, scale=LNSCALE)
                act(v12(LM, off, [1, n + 3]), v12(U, off, [1, n + 3]), F.Ln,
                    bias=1.0, scale=-LNSCALE)
                vec.tensor_tensor(v12(CV, off, [1, n + 3]),
                                  v12(LP, off, [1, n + 3]),
                                  v12(LM, off, [1, n + 3]), Op.subtract)

                # v6 leave-one-out sums of the d-role c2vs, depth 2:
                # X[0]=c10+c11  X[1]=c9+c11  X[2]=c9+c10
                X = work.tile([P, W * 3], f16, tag=f"X{c}", name="X")
                if not last:
                    # X[0], X[1] feed only the mid-iteration v6 m'-update
                    vec.tensor_tensor(v3(X, 0, [1, 2]), v12(CV, 10, [-1, 2]),
                                      v12(CV, 11, [0, 2]), Op.add)
                gps.tensor_tensor(v3(X, 2, [1, 1]), v12(CV, 9, [1, 1]),
                                  v12(CV, 10, [1, 1]), Op.add)

                if not last:
                    # m' for the six deg-2 edges: llr + partner c2v
                    vec.tensor_tensor(v9(M, 0, [1, 6]), v6(LB, 0, [1, 6]),
                                      v12(CV, 4, [2, 3], [-1, 2]), Op.add)
                    # m' for v6 edges: llr6 + sum of the other two c2v_d
                    vec.tensor_tensor(v9(M, 6, [1, 3]), v3(L6s[c], 0, [1, 3]),
                                      v3(X, 0, [1, 3]), Op.add)
                else:
                    # new_llr in natural variable order
                    SP = work.tile([P, W * 3], f32, tag=f"SP{c}", name="SP")
                    gps.tensor_tensor(v7(NL, 0, [1, 2]), v7(LL, 0, [1, 2]),
                                      v12(CV, 0, [1, 2]), Op.add)
                    gps.tensor_tensor(v7(NL, 3, [1, 1]), v7(LL, 3, [1, 1]),
                                      v12(CV, 2, [1, 1]), Op.add)
                    vec.tensor_tensor(SP[:], v12(CV, 3, [2, 3]),
                                      v12(CV, 4, [2, 3]), Op.add)
                    vec.tensor_tensor(v7(NL, 2, [1, 1]), v7(LL, 2, [1, 1]),
                                      v3(SP, 0, [1, 1]), Op.add)
                    vec.tensor_tensor(v7(NL, 4, [1, 2]), v7(LL, 4, [1, 2]),
                                      v3(SP, 1, [1, 2]), Op.add)
                    S1 = work.tile([P, W], f32, tag=f"S1{c}", name="S1")
                    vec.tensor_tensor(S1[:], v12(CV, 11, [1, 1]),
                                      v7(LL, 6, [1, 1]), Op.add)
                    vec.tensor_tensor(v7(NL, 6, [1, 1]), v3(X, 2, [1, 1]),
                                      S1[:], Op.add)
                    wl = W // 2
                    wh = W - wl
                    lo = bass.AP(tensor=NL[:].tensor, offset=NL[:].offset,
                                 ap=[list(NL[:].ap[0])] + [[7, wl], [1, 7]])
                    hi = bass.AP(tensor=NL[:].tensor,
                                 offset=NL[:].offset + wl * 7,
                                 ap=[list(NL[:].ap[0])] + [[7, wh], [1, 7]])
                    e0, e1 = ((nc.sync, nc.sync) if c < CHUNKS - 1
                              else (nc.sync, nc.gpsimd))
                    e0.dma_start(out=dram_view(out_d, c, 0, wl), in_=lo)
                    e1.dma_start(out=dram_view(out_d, c, wl, wh), in_=hi)

            # software-pipelined schedule: chunk 1 runs half an iteration
            # behind chunk 0 so each chunk's ACT phase (Tanh / Ln Ln) overlaps
            # the other chunk's vector phase (products / updates).  The
            # wait-until timestamps steer the Tile list scheduler into that
            # stagger; they are lower bounds only, data deps still rule.
            S0, HALF, GAP = _SCHED
            for it in range(iters):
                for c in range(CHUNKS):
                    tA = S0 + (CHUNKS * it + c) * HALF
                    with tc.tile_wait_until(tA / 1e6, enable=tA > 0):
                        partA(c, it)
                    with tc.tile_wait_until((tA + GAP) / 1e6):
                        partB(c, it)

    _strip_syncs(nc)
    return nc


def _strip_syncs(nc):
    """walrus on this stack supports a single sync-wait slot per instruction.
    Reduce each instruction's wait list via a vector-clock pass: walking the
    scheduled program order, every engine accumulates knowledge of semaphore
    values - from its own queue position, from waits it has already performed,
    and transitively from the producer's knowledge snapshot at the awaited
    update.  A wait already implied by that knowledge is dropped.  Kernel-tail
    drains keep only their DMA wait (the per-engine drain + EVSEM butterfly
    that follows enforces engine completion)."""
    import bass_rust

    eng_sem = {"EngineType.DVE": "DVE_", "EngineType.Pool": "Pool_",
               "EngineType.Activation": "Activation_", "EngineType.PE": "PE_",
               "EngineType.SP": "SP_"}
    know = {e: {} for e in eng_sem}          # engine -> {sem: value}
    sem_hist = {}                            # sem -> list of (cum_value, snapshot)
    sem_cum = {}                             # sem -> cumulative inc so far

    # Sems that are ever decremented (barrier gather sems) are not monotone;
    # leave their waits untouched and keep them out of the knowledge model.
    nonmono = set()
    for b in nc.m.functions[0].blocks:
        for inst in b.instructions:
            si = inst.sync_info
            if si is not None:
                for u in si.on_update:
                    if u.update_mode != "sem-inc":
                        nonmono.add(u.ant_name)

    def implied(k, sem, val):
        return k.get(sem, 0) >= val

    def learn(k, sem, val):
        if k.get(sem, 0) < val:
            k[sem] = val
        # transitively absorb the producer's snapshot at this update
        hist = sem_hist.get(sem)
        if hist:
            import bisect
            i = bisect.bisect_left([h[0] for h in hist], val)
            if i < len(hist):
                for s2, v2 in hist[i][1].items():
                    if k.get(s2, 0) < v2:
                        k[s2] = v2

    from concourse import mybir

    for b in nc.m.functions[0].blocks:
        new_instructions = []
        for inst in b.instructions:
            si = inst.sync_info
            eng = str(inst.engine)
            k = know.setdefault(eng, {})
            if si is not None:
                waits = list(si.on_wait)
                if type(inst).__name__ == "InstDrain" and len(waits) > 1:
                    dma = [w for w in waits if "DMA" in w.ant_name]
                    keep_w = dma[-1:] if dma else waits[:1]
                    for w in waits:
                        learn(k, w.ant_name, w.wait_value)
                else:
                    merged = {}
                    for w in waits:
                        if w.ant_name in nonmono:
                            merged[id(w)] = w
                        elif w.ant_name not in merged or \
                                merged[w.ant_name].wait_value < w.wait_value:
                            merged[w.ant_name] = w
                    keep_w = []
                    for w in merged.values():
                        if w.ant_name in nonmono:
                            keep_w.append(w)
                            continue
                        if not implied(k, w.ant_name, w.wait_value):
                            keep_w.append(w)
                        learn(k, w.ant_name, w.wait_value)
                    # walrus has one wait slot per instruction: hoist extra
                    # waits onto injected no-ops on the same engine
                    while len(keep_w) > 1:
                        w = keep_w.pop(0)
                        nop = mybir.InstNoOp(
                            name=f"{inst.name}_w{len(keep_w)}",
                            engine=inst.engine, ins=[], outs=[],
                            sync_info=bass_rust.SyncInfo(
                                on_wait=[w], on_update=[]))
                        new_instructions.append(nop)
                if len(keep_w) != len(waits):
                    inst.sync_info = bass_rust.SyncInfo(
                        on_wait=keep_w, on_update=list(si.on_update))
                    si = inst.sync_info
                for u in si.on_update:
                    if u.update_mode == "sem-inc" and u.ant_name not in nonmono:
                        name = u.ant_name
                        cum = sem_cum.get(name, 0) + u.update_value
                        sem_cum[name] = cum
                        # own-engine sems are implicitly ordered for later
                        # instructions on the same queue
                        pref = eng_sem.get(eng)
                        if pref and name.startswith(pref):
                            k[name] = max(k.get(name, 0), cum)
                        sem_hist.setdefault(name, []).append((cum, dict(k)))
            new_instructions.append(inst)
        if len(new_instructions) != len(b.instructions):
            b.instructions = new_instructions


def kernel(llr, max_iters):
    llr = np.ascontiguousarray(np.asarray(llr), dtype=np.float32)
    iters = int(np.asarray(max_iters))
    B = llr.shape[0]
    if iters <= 0:
        return llr.reshape(B, 1, 7).copy()

    from concourse.bass_utils import run_bass_kernel_spmd

    Bc = B // NCORES
    key = (Bc, iters)
    if key not in _CACHE:
        _CACHE[key] = _build(Bc, iters)
    nc = _CACHE[key]

    flat = llr.reshape(B, 7)
    in_maps = [{"llr": flat[i * Bc:(i + 1) * Bc]} for i in range(NCORES)]
    res = run_bass_kernel_spmd(nc, in_maps, core_ids=list(range(NCORES)))
    out = np.concatenate([np.asarray(r["out"]) for r in res.results], axis=0)
    return out.reshape(B, 1, 7)


# revision 50
# speedup vs baseline: 1.0078x; 1.0019x over previous
"""LDPC belief-propagation kernel for Trainium2 (8 NeuronCores, data-parallel).

Math (per batch row, H fixed [3,7], 12 edges):
  t_e   = tanh(m_e / 2)                       (signed!)
  u_e   = prod_{e' in check c, e' != e} t_e'  (signed leave-one-out product)
  c2v_e = 2 atanh(u_e) = ln(1+u) - ln(1-u)    (signed, sign handled for free)
  new_llr_v = llr_v + sum_{c contains v} c2v_{c,v}
  m'_e  = new_llr_v - c2v_e
Only Tanh/Ln tables; the sign pipeline of the classic phi/phi formulation
disappears because the tanh products carry signs natively.

Edge layout is role-major per 12-slot group: [s0 s1 s2 | a0 a1 a2 | b0 b1 b2 |
d0 d1 d2] where s_c is the check's degree-1-variable edge (v0,v1,v3 - their
messages never change), d_c is v6's edge in check c, and (a, b) =
((v2c0, v2c1, v4c2), (v4c0, v5c1, v5c2)).  This makes every structural op a
single strided instruction:
  Q[k]   = T[k] * T[k+6]         (k=0..5: pair products (s*b, a*d) per check)
  U[3..8]  = T[(9,10,11,0,1,2)] * Q[0..5]   (loo for roles a, b)
  U[9..11] = T[3..5] * Q[0..2]              (loo for role d)
  M'[deg2 six edges] = LB6 + CV[partner]    (partner = pair-swap view)
  M'[d]  = (c2v_d-sum + llr_6) - CV[d]      (v6 leave-one-out via total sum)
Batch is split into 3 width-tuned chunks (84/88/84 columns per partition)
whose iterations run staggered, so ACT (Tanh + the two Ln ops), DVE
(products, c2v subtract, message updates) and Pool (d-role products, v6
pair-sums) stay concurrently busy; iteration 0 reads tanh(llr) directly so
the edge-slot scatter stays off the pipeline-fill critical path, and
setup copies / output DMAs are placed by schedule hints to keep the fill
and drain edges tight.  Sync info is reduced to walrus's one-wait-slot
limit by a vector-clock pass (_strip_syncs).
"""

import numpy as np

_CACHE = {}

NCORES = 8
P = 128      # partitions
CHUNKS = 3   # batch sub-chunks per core (pipeline depth)

# guard so ln(1 -+ 0.99999988*u) stays finite (>= ~1.2e-7) even at u = -+1
LNSCALE = 0.99999988

# (start, half-period, partA->partB gap) ns hints for the list scheduler
_SCHED = (0, 0, 0)

# schedule hints deferring chunk setup copies (ns)
_CPY = (0, 9800, 12300)

# manual chunk widths (must sum to Bc//P//1); None = near-even
_WS = (86, 88, 82)


def _build(Bc, iters):
    import contextlib

    import concourse.bass as bass
    import concourse.tile as tile
    from concourse import mybir
    from concourse.alu_op_type import AluOpType as Op

    F = mybir.ActivationFunctionType
    Wtot = Bc // P
    if _WS is not None and sum(_WS) == Wtot:
        Ws = list(_WS)
    else:
        base, rem = divmod(Wtot, CHUNKS)
        Ws = [base + (1 if i == 1 else 0) for i in range(CHUNKS)] \
            if rem == 1 else \
            [base + (1 if i >= CHUNKS - rem else 0) for i in range(CHUNKS)]
    f32 = mybir.dt.float32

    f16 = mybir.dt.float16
    nc = bass.Bass("TRN2", target_bir_lowering=False, debug=False,
                   num_devices=1)
    llr_d = nc.dram_tensor("llr", [Bc, 7], f32, kind="ExternalInput")
    out_d = nc.dram_tensor("out", [Bc, 7], f32, kind="ExternalOutput")

    def sub(t, off, dims):
        a = t[:] if callable(getattr(t, "__getitem__", None)) else t
        return bass.AP(tensor=a.tensor, offset=a.offset + off,
                       ap=[list(a.ap[0])] + [list(d) for d in dims])

    with tile.TileContext(nc) as tc:
        ctx = contextlib.ExitStack()
        with ctx:
            keep = ctx.enter_context(tc.tile_pool(name="keep", bufs=1))
            work = ctx.enter_context(tc.tile_pool(name="work", bufs=2))

            def K(name, c, k, dt=f32):
                return keep.tile([P, Ws[c] * k], dt, tag=name, name=name)

            # per-chunk persistent state
            LLs = [K(f"LL{c}", c, 7) for c in range(CHUNKS)]    # llr, natural v order
            LBs = [K(f"LB{c}", c, 6, f16) for c in range(CHUNKS)]   # llr bcast, deg2 edges
            L6s = [K(f"L6{c}", c, 3, f16) for c in range(CHUNKS)]   # llr6 bcast, v6 edges
            Ts  = [K(f"T{c}", c, 12) for c in range(CHUNKS)]    # tanh(m/2) per edge
            Ms  = [K(f"M{c}", c, 9, f16) for c in range(CHUNKS)]    # dyn messages
            NLs = [K(f"NL{c}", c, 7) for c in range(CHUNKS)]    # output llr

            act = nc.scalar.activation
            vec = nc.vector
            gps = nc.gpsimd

            def dram_view(t, c, w0, nw):
                # [P, nw*7] window of chunk c: rows base_c + p*Ws[c] + w
                a = t.ap()
                off = (P * sum(Ws[:c]) + w0) * 7
                return bass.AP(tensor=a.tensor, offset=a.offset + off,
                               ap=[[Ws[c] * 7, P], [1, nw * 7]])

            for c in range(CHUNKS):
                eng = nc.sync if c == 0 else nc.gpsimd
                eng.dma_start(out=LLs[c][:], in_=dram_view(llr_d, c, 0, Ws[c]))

            cur = {"W": Ws[0]}

            def v7(t, off, *dims):
                return sub(t, off, [[7, cur["W"]]] + [list(d) for d in dims])

            def v12(t, off, *dims):
                return sub(t, off, [[12, cur["W"]]] + [list(d) for d in dims])

            def v9(t, off, *dims):
                return sub(t, off, [[9, cur["W"]]] + [list(d) for d in dims])

            def v6(t, off, *dims):
                return sub(t, off, [[6, cur["W"]]] + [list(d) for d in dims])

            def v3(t, off, *dims):
                return sub(t, off, [[3, cur["W"]]] + [list(d) for d in dims])

            state = [{} for _ in range(CHUNKS)]

            def partA(c, it):
                """tanh + products: T, Q, U."""
                LL, LB, T, M = LLs[c], LBs[c], Ts[c], Ms[c]
                last = (it == iters - 1)
                W = Ws[c]
                cur["W"] = W
                Q = work.tile([P, W * 6], f32, tag=f"Q{c}", name="Q")
                U = work.tile([P, W * 12], f32, tag=f"U{c}", name="U")
                state[c] = {"Q": Q, "U": U}

                if it == 0:
                    # t = tanh(llr/2) once; iteration-0 products read TL
                    # directly so the T scatter stays off the critical path
                    TL = work.tile([P, W * 7], f32, tag=f"TL{c}", name="TL")
                    act(TL[:], LL[:], F.Tanh, scale=0.5)
                    vec.tensor_tensor(v6(Q, 0, [1, 2]), v7(TL, 0, [1, 2]),
                                      v7(TL, 4, [1, 2]), Op.mult)
                    vec.tensor_tensor(v6(Q, 2, [1, 1]), v7(TL, 3, [1, 1]),
                                      v7(TL, 5, [1, 1]), Op.mult)
                    gps.tensor_tensor(v6(Q, 3, [1, 2]), v7(TL, 2, [0, 2]),
                                      v7(TL, 6, [0, 2]), Op.mult)
                    gps.tensor_tensor(v6(Q, 5, [1, 1]), v7(TL, 4, [1, 1]),
                                      v7(TL, 6, [1, 1]), Op.mult)
                    vec.tensor_tensor(v12(U, 3, [1, 3]), v7(TL, 6, [0, 3]),
                                      v6(Q, 0, [1, 3]), Op.mult)
                    vec.tensor_tensor(v12(U, 6, [1, 2]), v7(TL, 0, [1, 2]),
                                      v6(Q, 3, [1, 2]), Op.mult)
                    vec.tensor_tensor(v12(U, 8, [1, 1]), v7(TL, 3, [1, 1]),
                                      v6(Q, 5, [1, 1]), Op.mult)
                    gps.tensor_tensor(v12(U, 9, [1, 2]), v7(TL, 2, [0, 2]),
                                      v6(Q, 0, [1, 2]), Op.mult)
                    gps.tensor_tensor(v12(U, 11, [1, 1]), v7(TL, 4, [1, 1]),
                                      v6(Q, 2, [1, 1]), Op.mult)
                    if last:  # iters == 1
                        vec.tensor_tensor(v12(U, 0, [1, 2]), v7(TL, 4, [1, 2]),
                                          v6(Q, 3, [1, 2]), Op.mult)
                        vec.tensor_tensor(v12(U, 2, [1, 1]), v7(TL, 5, [1, 1]),
                                          v6(Q, 5, [1, 1]), Op.mult)
                    # scatter t to role-major slots for later iterations,
                    # off the critical path (only statics strictly needed
                    # before iteration 1's products)
                    vec.tensor_copy(v12(T, 0, [1, 2]), v7(TL, 0, [1, 2]))
                    vec.tensor_copy(v12(T, 2, [1, 1]), v7(TL, 3, [1, 1]))
                    with tc.tile_wait_until(_CPY[c] / 1e6, enable=_CPY[c] > 0):
                        gps.tensor_copy(v6(LB, 0, [1, 2]), v7(LL, 2, [0, 2]))
                        gps.tensor_copy(v6(LB, 2, [1, 4]),
                                        v7(LL, 4, [1, 2], [0, 2]))
                        gps.tensor_copy(v3(L6s[c], 0, [1, 3]),
                                        v7(LL, 6, [0, 3]))
                else:
                    act(v12(T, 3, [1, 9]), M[:], F.Tanh, scale=0.5)
                    # pair products and signed leave-one-out products
                    vec.tensor_tensor(Q[:], v12(T, 0, [1, 6]),
                                      v12(T, 6, [1, 6]), Op.mult)
                    vec.tensor_tensor(v12(U, 3, [1, 6]),
                                      v12(T, 9, [-9, 2], [1, 3]),
                                      v6(Q, 0, [1, 6]), Op.mult)
                    gps.tensor_tensor(v12(U, 9, [1, 3]), v12(T, 3, [1, 3]),
                                      v6(Q, 0, [1, 3]), Op.mult)
                    if last:
                        vec.tensor_tensor(v12(U, 0, [1, 3]), v12(T, 6, [1, 3]),
                                          v6(Q, 3, [1, 3]), Op.mult)

            def partB(c, it):
                """c2v + message/new-llr update."""
                LL, LB, M, NL = LLs[c], LBs[c], Ms[c], NLs[c]
                last = (it == iters - 1)
                W = Ws[c]
                cur["W"] = W
                U = state[c]["U"]
                LP = work.tile([P, W * 12], f16, tag=f"LP{c}", name="LP")
                LM = work.tile([P, W * 12], f16, tag=f"LM{c}", name="LM")
                CV = work.tile([P, W * 12], f16, tag=f"CV{c}", name="CV")

                off, n = (0, 9) if last else (3, 6)
                # c2v = ln(1+u) - ln(1-u), guarded away from ln(0)
                act(v12(LP, off, [1, n + 3]), v12(U, off, [1, n + 3]), F.Ln,
                    bias=1.0, scale=LNSCALE)
                act(v12(LM, off, [1, n + 3]), v12(U, off, [1, n + 3]), F.Ln,
                    bias=1.0, scale=-LNSCALE)
                vec.tensor_tensor(v12(CV, off, [1, n + 3]),
                                  v12(LP, off, [1, n + 3]),
                                  v12(LM, off, [1, n + 3]), Op.subtract)

                # v6 leave-one-out sums of the d-role c2vs, depth 2:
                # X[0]=c10+c11  X[1]=c9+c11  X[2]=c9+c10
                X = work.tile([P, W * 3], f16, tag=f"X{c}", name="X")
                if not last:
                    # X[0], X[1] feed only the mid-iteration v6 m'-update
                    vec.tensor_tensor(v3(X, 0, [1, 2]), v12(CV, 10, [-1, 2]),
                                      v12(CV, 11, [0, 2]), Op.add)
                gps.tensor_tensor(v3(X, 2, [1, 1]), v12(CV, 9, [1, 1]),
                                  v12(CV, 10, [1, 1]), Op.add)

                if not last:
                    # m' for the six deg-2 edges: llr + partner c2v
                    vec.tensor_tensor(v9(M, 0, [1, 6]), v6(LB, 0, [1, 6]),
                                      v12(CV, 4, [2, 3], [-1, 2]), Op.add)
                    # m' for v6 edges: llr6 + sum of the other two c2v_d
                    vec.tensor_tensor(v9(M, 6, [1, 3]), v3(L6s[c], 0, [1, 3]),
                                      v3(X, 0, [1, 3]), Op.add)
                else:
                    # new_llr in natural variable order
                    SP = work.tile([P, W * 3], f32, tag=f"SP{c}", name="SP")
                    gps.tensor_tensor(v7(NL, 0, [1, 2]), v7(LL, 0, [1, 2]),
                                      v12(CV, 0, [1, 2]), Op.add)
                    gps.tensor_tensor(v7(NL, 3, [1, 1]), v7(LL, 3, [1, 1]),
                                      v12(CV, 2, [1, 1]), Op.add)
                    vec.tensor_tensor(SP[:], v12(CV, 3, [2, 3]),
                                      v12(CV, 4, [2, 3]), Op.add)
                    vec.tensor_tensor(v7(NL, 2, [1, 1]), v7(LL, 2, [1, 1]),
                                      v3(SP, 0, [1, 1]), Op.add)
                    vec.tensor_tensor(v7(NL, 4, [1, 2]), v7(LL, 4, [1, 2]),
                                      v3(SP, 1, [1, 2]), Op.add)
                    S1 = work.tile([P, W], f32, tag=f"S1{c}", name="S1")
                    vec.tensor_tensor(S1[:], v12(CV, 11, [1, 1]),
                                      v7(LL, 6, [1, 1]), Op.add)
                    vec.tensor_tensor(v7(NL, 6, [1, 1]), v3(X, 2, [1, 1]),
                                      S1[:], Op.add)
                    wl = W // 2
                    wh = W - wl
                    lo = bass.AP(tensor=NL[:].tensor, offset=NL[:].offset,
                                 ap=[list(NL[:].ap[0])] + [[7, wl], [1, 7]])
                    hi = bass.AP(tensor=NL[:].tensor,
                                 offset=NL[:].offset + wl * 7,
                                 ap=[list(NL[:].ap[0])] + [[7, wh], [1, 7]])
                    e0, e1 = ((nc.sync, nc.sync) if c < CHUNKS - 1
                              else (nc.sync, nc.gpsimd))
                    e0.dma_start(out=dram_view(out_d, c, 0, wl), in_=lo)
                    e1.dma_start(out=dram_view(out_d, c, wl, wh), in_=hi)

            # software-pipelined schedule: chunk 1 runs half an iteration
            # behind chunk 0 so each chunk's ACT phase (Tanh / Ln Ln) overlaps
            # the other chunk's vector phase (products / updates).  The
            # wait-until timestamps steer the Tile list scheduler into that
            # stagger; they are lower bounds only, data deps still rule.
            S0, HALF, GAP = _SCHED
            for it in range(iters):
                for c in range(CHUNKS):
                    tA = S0 + (CHUNKS * it + c) * HALF
                    with tc.tile_wait_until(tA / 1e6, enable=tA > 0):
                        partA(c, it)
                    with tc.tile_wait_until((tA + GAP) / 1e6):
                        partB(c, it)

    _strip_syncs(nc)
    return nc


def _strip_syncs(nc):
    """walrus on this stack supports a single sync-wait slot per instruction.
    Reduce each instruction's wait list via a vector-clock pass: walking the
    scheduled program order, every engine accumulates knowledge of semaphore
    values - from its own queue position, from waits it has already performed,
    and transitively from the producer's knowledge snapshot at the awaited
    update.  A wait already implied by that knowledge is dropped.  Kernel-tail
    drains keep only their DMA wait (the per-engine drain + EVSEM butterfly
    that follows enforces engine completion)."""
    import bass_rust

    eng_sem = {"EngineType.DVE": "DVE_", "EngineType.Pool": "Pool_",
               "EngineType.Activation": "Activation_", "EngineType.PE": "PE_",
               "EngineType.SP": "SP_"}
    know = {e: {} for e in eng_sem}          # engine -> {sem: value}
    sem_hist = {}                            # sem -> list of (cum_value, snapshot)
    sem_cum = {}                             # sem -> cumulative inc so far

    # Sems that are ever decremented (barrier gather sems) are not monotone;
    # leave their waits untouched and keep them out of the knowledge model.
    nonmono = set()
    for b in nc.m.functions[0].blocks:
        for inst in b.instructions:
            si = inst.sync_info
            if si is not None:
                for u in si.on_update:
                    if u.update_mode != "sem-inc":
                        nonmono.add(u.ant_name)

    def implied(k, sem, val):
        return k.get(sem, 0) >= val

    def learn(k, sem, val):
        if k.get(sem, 0) < val:
            k[sem] = val
        # transitively absorb the producer's snapshot at this update
        hist = sem_hist.get(sem)
        if hist:
            import bisect
            i = bisect.bisect_left([h[0] for h in hist], val)
            if i < len(hist):
                for s2, v2 in hist[i][1].items():
                    if k.get(s2, 0) < v2:
                        k[s2] = v2

    from concourse import mybir

    for b in nc.m.functions[0].blocks:
        new_instructions = []
        for inst in b.instructions:
            si = inst.sync_info
            eng = str(inst.engine)
            k = know.setdefault(eng, {})
            if si is not None:
                waits = list(si.on_wait)
                if type(inst).__name__ == "InstDrain" and len(waits) > 1:
                    dma = [w for w in waits if "DMA" in w.ant_name]
                    keep_w = dma[-1:] if dma else waits[:1]
                    for w in waits:
                        learn(k, w.ant_name, w.wait_value)
                else:
                    merged = {}
                    for w in waits:
                        if w.ant_name in nonmono:
                            merged[id(w)] = w
                        elif w.ant_name not in merged or \
                                merged[w.ant_name].wait_value < w.wait_value:
                            merged[w.ant_name] = w
                    keep_w = []
                    for w in merged.values():
                        if w.ant_name in nonmono:
                            keep_w.append(w)
                            continue
                        if not implied(k, w.ant_name, w.wait_value):
                            keep_w.append(w)
                        learn(k, w.ant_name, w.wait_value)
                    # walrus has one wait slot per instruction: hoist extra
                    # waits onto injected no-ops on the same engine
                    while len(keep_w) > 1:
                        w = keep_w.pop(0)
                        nop = mybir.InstNoOp(
                            name=f"{inst.name}_w{len(keep_w)}",
                            engine=inst.engine, ins=[], outs=[],
                            sync_info=bass_rust.SyncInfo(
                                on_wait=[w], on_update=[]))
                        new_instructions.append(nop)
                if len(keep_w) != len(waits):
                    inst.sync_info = bass_rust.SyncInfo(
                        on_wait=keep_w, on_update=list(si.on_update))
                    si = inst.sync_info
                for u in si.on_update:
                    if u.update_mode == "sem-inc" and u.ant_name not in nonmono:
                        name = u.ant_name
                        cum = sem_cum.get(name, 0) + u.update_value
                        sem_cum[name] = cum
                        # own-engine sems are implicitly ordered for later
                        # instructions on the same queue
                        pref = eng_sem.get(eng)
                        if pref and name.startswith(pref):
                            k[name] = max(k.get(name, 0), cum)
                        sem_hist.setdefault(name, []).append((cum, dict(k)))
            new_instructions.append(inst)
        if len(new_instructions) != len(b.instructions):
            b.instructions = new_instructions


def kernel(llr, max_iters):
    llr = np.ascontiguousarray(np.asarray(llr), dtype=np.float32)
    iters = int(np.asarray(max_iters))
    B = llr.shape[0]
    if iters <= 0:
        return llr.reshape(B, 1, 7).copy()

    from concourse.bass_utils import run_bass_kernel_spmd

    Bc = B // NCORES
    key = (Bc, iters)
    if key not in _CACHE:
        _CACHE[key] = _build(Bc, iters)
    nc = _CACHE[key]

    flat = llr.reshape(B, 7)
    in_maps = [{"llr": flat[i * Bc:(i + 1) * Bc]} for i in range(NCORES)]
    res = run_bass_kernel_spmd(nc, in_maps, core_ids=list(range(NCORES)))
    out = np.concatenate([np.asarray(r["out"]) for r in res.results], axis=0)
    return out.reshape(B, 1, 7)


# revision 51
# speedup vs baseline: 1.0145x; 1.0067x over previous
"""LDPC belief-propagation kernel for Trainium2 (8 NeuronCores, data-parallel).

Math (per batch row, H fixed [3,7], 12 edges):
  t_e   = tanh(m_e / 2)                       (signed!)
  u_e   = prod_{e' in check c, e' != e} t_e'  (signed leave-one-out product)
  c2v_e = 2 atanh(u_e) = ln(1+u) - ln(1-u)    (signed, sign handled for free)
  new_llr_v = llr_v + sum_{c contains v} c2v_{c,v}
  m'_e  = new_llr_v - c2v_e
Only Tanh/Ln tables; the sign pipeline of the classic phi/phi formulation
disappears because the tanh products carry signs natively.

Edge layout is role-major per 12-slot group: [s0 s1 s2 | a0 a1 a2 | b0 b1 b2 |
d0 d1 d2] where s_c is the check's degree-1-variable edge (v0,v1,v3 - their
messages never change), d_c is v6's edge in check c, and (a, b) =
((v2c0, v2c1, v4c2), (v4c0, v5c1, v5c2)).  This makes every structural op a
single strided instruction:
  Q[k]   = T[k] * T[k+6]         (k=0..5: pair products (s*b, a*d) per check)
  U[3..8]  = T[(9,10,11,0,1,2)] * Q[0..5]   (loo for roles a, b)
  U[9..11] = T[3..5] * Q[0..2]              (loo for role d)
  M'[deg2 six edges] = LB6 + CV[partner]    (partner = pair-swap view)
  M'[d]  = (c2v_d-sum + llr_6) - CV[d]      (v6 leave-one-out via total sum)
Batch is split into 3 width-tuned chunks (88/90/78 columns per partition)
whose iterations run staggered, so ACT (Tanh + the two Ln ops), DVE
(products, c2v subtract, message updates) and Pool (d-role products, v6
pair-sums) stay concurrently busy; iteration 0 reads tanh(llr) directly so
the edge-slot scatter stays off the pipeline-fill critical path, and
setup copies / output DMAs are placed by schedule hints to keep the fill
and drain edges tight.  Sync info is reduced to walrus's one-wait-slot
limit by a vector-clock pass (_strip_syncs).
"""

import numpy as np

_CACHE = {}

NCORES = 8
P = 128      # partitions
CHUNKS = 3   # batch sub-chunks per core (pipeline depth)

# guard so ln(1 -+ 0.99999988*u) stays finite (>= ~1.2e-7) even at u = -+1
LNSCALE = 0.99999988

# (start, half-period, partA->partB gap) ns hints for the list scheduler
_SCHED = (0, 0, 0)

# schedule hints deferring chunk setup copies (ns)
_CPY = (0, 9800, 12300)

# manual chunk widths (must sum to Bc//P//1); None = near-even
_WS = (88, 90, 78)


def _build(Bc, iters):
    import contextlib

    import concourse.bass as bass
    import concourse.tile as tile
    from concourse import mybir
    from concourse.alu_op_type import AluOpType as Op

    F = mybir.ActivationFunctionType
    Wtot = Bc // P
    if _WS is not None and sum(_WS) == Wtot:
        Ws = list(_WS)
    else:
        base, rem = divmod(Wtot, CHUNKS)
        Ws = [base + (1 if i == 1 else 0) for i in range(CHUNKS)] \
            if rem == 1 else \
            [base + (1 if i >= CHUNKS - rem else 0) for i in range(CHUNKS)]
    f32 = mybir.dt.float32

    f16 = mybir.dt.float16
    nc = bass.Bass("TRN2", target_bir_lowering=False, debug=False,
                   num_devices=1)
    llr_d = nc.dram_tensor("llr", [Bc, 7], f32, kind="ExternalInput")
    out_d = nc.dram_tensor("out", [Bc, 7], f32, kind="ExternalOutput")

    def sub(t, off, dims):
        a = t[:] if callable(getattr(t, "__getitem__", None)) else t
        return bass.AP(tensor=a.tensor, offset=a.offset + off,
                       ap=[list(a.ap[0])] + [list(d) for d in dims])

    with tile.TileContext(nc) as tc:
        ctx = contextlib.ExitStack()
        with ctx:
            keep = ctx.enter_context(tc.tile_pool(name="keep", bufs=1))
            work = ctx.enter_context(tc.tile_pool(name="work", bufs=2))

            def K(name, c, k, dt=f32):
                return keep.tile([P, Ws[c] * k], dt, tag=name, name=name)

            # per-chunk persistent state
            LLs = [K(f"LL{c}", c, 7) for c in range(CHUNKS)]    # llr, natural v order
            LBs = [K(f"LB{c}", c, 6, f16) for c in range(CHUNKS)]   # llr bcast, deg2 edges
            L6s = [K(f"L6{c}", c, 3, f16) for c in range(CHUNKS)]   # llr6 bcast, v6 edges
            Ts  = [K(f"T{c}", c, 12) for c in range(CHUNKS)]    # tanh(m/2) per edge
            Ms  = [K(f"M{c}", c, 9, f16) for c in range(CHUNKS)]    # dyn messages
            NLs = [K(f"NL{c}", c, 7) for c in range(CHUNKS)]    # output llr

            act = nc.scalar.activation
            vec = nc.vector
            gps = nc.gpsimd

            def dram_view(t, c, w0, nw):
                # [P, nw*7] window of chunk c: rows base_c + p*Ws[c] + w
                a = t.ap()
                off = (P * sum(Ws[:c]) + w0) * 7
                return bass.AP(tensor=a.tensor, offset=a.offset + off,
                               ap=[[Ws[c] * 7, P], [1, nw * 7]])

            for c in range(CHUNKS):
                eng = nc.sync if c == 0 else nc.gpsimd
                eng.dma_start(out=LLs[c][:], in_=dram_view(llr_d, c, 0, Ws[c]))

            cur = {"W": Ws[0]}

            def v7(t, off, *dims):
                return sub(t, off, [[7, cur["W"]]] + [list(d) for d in dims])

            def v12(t, off, *dims):
                return sub(t, off, [[12, cur["W"]]] + [list(d) for d in dims])

            def v9(t, off, *dims):
                return sub(t, off, [[9, cur["W"]]] + [list(d) for d in dims])

            def v6(t, off, *dims):
                return sub(t, off, [[6, cur["W"]]] + [list(d) for d in dims])

            def v3(t, off, *dims):
                return sub(t, off, [[3, cur["W"]]] + [list(d) for d in dims])

            state = [{} for _ in range(CHUNKS)]

            def partA(c, it):
                """tanh + products: T, Q, U."""
                LL, LB, T, M = LLs[c], LBs[c], Ts[c], Ms[c]
                last = (it == iters - 1)
                W = Ws[c]
                cur["W"] = W
                Q = work.tile([P, W * 6], f32, tag=f"Q{c}", name="Q")
                U = work.tile([P, W * 12], f32, tag=f"U{c}", name="U")
                state[c] = {"Q": Q, "U": U}

                if it == 0:
                    # t = tanh(llr/2) once; iteration-0 products read TL
                    # directly so the T scatter stays off the critical path
                    TL = work.tile([P, W * 7], f32, tag=f"TL{c}", name="TL")
                    act(TL[:], LL[:], F.Tanh, scale=0.5)
                    vec.tensor_tensor(v6(Q, 0, [1, 2]), v7(TL, 0, [1, 2]),
                                      v7(TL, 4, [1, 2]), Op.mult)
                    vec.tensor_tensor(v6(Q, 2, [1, 1]), v7(TL, 3, [1, 1]),
                                      v7(TL, 5, [1, 1]), Op.mult)
                    gps.tensor_tensor(v6(Q, 3, [1, 2]), v7(TL, 2, [0, 2]),
                                      v7(TL, 6, [0, 2]), Op.mult)
                    gps.tensor_tensor(v6(Q, 5, [1, 1]), v7(TL, 4, [1, 1]),
                                      v7(TL, 6, [1, 1]), Op.mult)
                    vec.tensor_tensor(v12(U, 3, [1, 3]), v7(TL, 6, [0, 3]),
                                      v6(Q, 0, [1, 3]), Op.mult)
                    vec.tensor_tensor(v12(U, 6, [1, 2]), v7(TL, 0, [1, 2]),
                                      v6(Q, 3, [1, 2]), Op.mult)
                    vec.tensor_tensor(v12(U, 8, [1, 1]), v7(TL, 3, [1, 1]),
                                      v6(Q, 5, [1, 1]), Op.mult)
                    gps.tensor_tensor(v12(U, 9, [1, 2]), v7(TL, 2, [0, 2]),
                                      v6(Q, 0, [1, 2]), Op.mult)
                    gps.tensor_tensor(v12(U, 11, [1, 1]), v7(TL, 4, [1, 1]),
                                      v6(Q, 2, [1, 1]), Op.mult)
                    if last:  # iters == 1
                        vec.tensor_tensor(v12(U, 0, [1, 2]), v7(TL, 4, [1, 2]),
                                          v6(Q, 3, [1, 2]), Op.mult)
                        vec.tensor_tensor(v12(U, 2, [1, 1]), v7(TL, 5, [1, 1]),
                                          v6(Q, 5, [1, 1]), Op.mult)
                    # scatter t to role-major slots for later iterations,
                    # off the critical path (only statics strictly needed
                    # before iteration 1's products)
                    vec.tensor_copy(v12(T, 0, [1, 2]), v7(TL, 0, [1, 2]))
                    vec.tensor_copy(v12(T, 2, [1, 1]), v7(TL, 3, [1, 1]))
                    with tc.tile_wait_until(_CPY[c] / 1e6, enable=_CPY[c] > 0):
                        gps.tensor_copy(v6(LB, 0, [1, 2]), v7(LL, 2, [0, 2]))
                        gps.tensor_copy(v6(LB, 2, [1, 4]),
                                        v7(LL, 4, [1, 2], [0, 2]))
                        gps.tensor_copy(v3(L6s[c], 0, [1, 3]),
                                        v7(LL, 6, [0, 3]))
                else:
                    act(v12(T, 3, [1, 9]), M[:], F.Tanh, scale=0.5)
                    # pair products and signed leave-one-out products
                    vec.tensor_tensor(Q[:], v12(T, 0, [1, 6]),
                                      v12(T, 6, [1, 6]), Op.mult)
                    vec.tensor_tensor(v12(U, 3, [1, 6]),
                                      v12(T, 9, [-9, 2], [1, 3]),
                                      v6(Q, 0, [1, 6]), Op.mult)
                    gps.tensor_tensor(v12(U, 9, [1, 3]), v12(T, 3, [1, 3]),
                                      v6(Q, 0, [1, 3]), Op.mult)
                    if last:
                        vec.tensor_tensor(v12(U, 0, [1, 3]), v12(T, 6, [1, 3]),
                                          v6(Q, 3, [1, 3]), Op.mult)

            def partB(c, it):
                """c2v + message/new-llr update."""
                LL, LB, M, NL = LLs[c], LBs[c], Ms[c], NLs[c]
                last = (it == iters - 1)
                W = Ws[c]
                cur["W"] = W
                U = state[c]["U"]
                LP = work.tile([P, W * 12], f16, tag=f"LP{c}", name="LP")
                LM = work.tile([P, W * 12], f16, tag=f"LM{c}", name="LM")
                CV = work.tile([P, W * 12], f16, tag=f"CV{c}", name="CV")

                off, n = (0, 9) if last else (3, 6)
                # c2v = ln(1+u) - ln(1-u), guarded away from ln(0)
                act(v12(LP, off, [1, n + 3]), v12(U, off, [1, n + 3]), F.Ln,
                    bias=1.0, scale=LNSCALE)
                act(v12(LM, off, [1, n + 3]), v12(U, off, [1, n + 3]), F.Ln,
                    bias=1.0, scale=-LNSCALE)
                vec.tensor_tensor(v12(CV, off, [1, n + 3]),
                                  v12(LP, off, [1, n + 3]),
                                  v12(LM, off, [1, n + 3]), Op.subtract)

                # v6 leave-one-out sums of the d-role c2vs, depth 2:
                # X[0]=c10+c11  X[1]=c9+c11  X[2]=c9+c10
                X = work.tile([P, W * 3], f16, tag=f"X{c}", name="X")
                if not last:
                    # X[0], X[1] feed only the mid-iteration v6 m'-update
                    vec.tensor_tensor(v3(X, 0, [1, 2]), v12(CV, 10, [-1, 2]),
                                      v12(CV, 11, [0, 2]), Op.add)
                gps.tensor_tensor(v3(X, 2, [1, 1]), v12(CV, 9, [1, 1]),
                                  v12(CV, 10, [1, 1]), Op.add)

                if not last:
                    # m' for the six deg-2 edges: llr + partner c2v
                    vec.tensor_tensor(v9(M, 0, [1, 6]), v6(LB, 0, [1, 6]),
                                      v12(CV, 4, [2, 3], [-1, 2]), Op.add)
                    # m' for v6 edges: llr6 + sum of the other two c2v_d
                    vec.tensor_tensor(v9(M, 6, [1, 3]), v3(L6s[c], 0, [1, 3]),
                                      v3(X, 0, [1, 3]), Op.add)
                else:
                    # new_llr in natural variable order
                    SP = work.tile([P, W * 3], f32, tag=f"SP{c}", name="SP")
                    gps.tensor_tensor(v7(NL, 0, [1, 2]), v7(LL, 0, [1, 2]),
                                      v12(CV, 0, [1, 2]), Op.add)
                    gps.tensor_tensor(v7(NL, 3, [1, 1]), v7(LL, 3, [1, 1]),
                                      v12(CV, 2, [1, 1]), Op.add)
                    vec.tensor_tensor(SP[:], v12(CV, 3, [2, 3]),
                                      v12(CV, 4, [2, 3]), Op.add)
                    vec.tensor_tensor(v7(NL, 2, [1, 1]), v7(LL, 2, [1, 1]),
                                      v3(SP, 0, [1, 1]), Op.add)
                    vec.tensor_tensor(v7(NL, 4, [1, 2]), v7(LL, 4, [1, 2]),
                                      v3(SP, 1, [1, 2]), Op.add)
                    S1 = work.tile([P, W], f32, tag=f"S1{c}", name="S1")
                    vec.tensor_tensor(S1[:], v12(CV, 11, [1, 1]),
                                      v7(LL, 6, [1, 1]), Op.add)
                    vec.tensor_tensor(v7(NL, 6, [1, 1]), v3(X, 2, [1, 1]),
                                      S1[:], Op.add)
                    wl = W // 2
                    wh = W - wl
                    lo = bass.AP(tensor=NL[:].tensor, offset=NL[:].offset,
                                 ap=[list(NL[:].ap[0])] + [[7, wl], [1, 7]])
                    hi = bass.AP(tensor=NL[:].tensor,
                                 offset=NL[:].offset + wl * 7,
                                 ap=[list(NL[:].ap[0])] + [[7, wh], [1, 7]])
                    e0, e1 = ((nc.sync, nc.sync) if c < CHUNKS - 1
                              else (nc.sync, nc.gpsimd))
                    e0.dma_start(out=dram_view(out_d, c, 0, wl), in_=lo)
                    e1.dma_start(out=dram_view(out_d, c, wl, wh), in_=hi)

            # software-pipelined schedule: chunk 1 runs half an iteration
            # behind chunk 0 so each chunk's ACT phase (Tanh / Ln Ln) overlaps
            # the other chunk's vector phase (products / updates).  The
            # wait-until timestamps steer the Tile list scheduler into that
            # stagger; they are lower bounds only, data deps still rule.
            S0, HALF, GAP = _SCHED
            for it in range(iters):
                for c in range(CHUNKS):
                    tA = S0 + (CHUNKS * it + c) * HALF
                    with tc.tile_wait_until(tA / 1e6, enable=tA > 0):
                        partA(c, it)
                    with tc.tile_wait_until((tA + GAP) / 1e6):
                        partB(c, it)

    _strip_syncs(nc)
    return nc


def _strip_syncs(nc):
    """walrus on this stack supports a single sync-wait slot per instruction.
    Reduce each instruction's wait list via a vector-clock pass: walking the
    scheduled program order, every engine accumulates knowledge of semaphore
    values - from its own queue position, from waits it has already performed,
    and transitively from the producer's knowledge snapshot at the awaited
    update.  A wait already implied by that knowledge is dropped.  Kernel-tail
    drains keep only their DMA wait (the per-engine drain + EVSEM butterfly
    that follows enforces engine completion)."""
    import bass_rust

    eng_sem = {"EngineType.DVE": "DVE_", "EngineType.Pool": "Pool_",
               "EngineType.Activation": "Activation_", "EngineType.PE": "PE_",
               "EngineType.SP": "SP_"}
    know = {e: {} for e in eng_sem}          # engine -> {sem: value}
    sem_hist = {}                            # sem -> list of (cum_value, snapshot)
    sem_cum = {}                             # sem -> cumulative inc so far

    # Sems that are ever decremented (barrier gather sems) are not monotone;
    # leave their waits untouched and keep them out of the knowledge model.
    nonmono = set()
    for b in nc.m.functions[0].blocks:
        for inst in b.instructions:
            si = inst.sync_info
            if si is not None:
                for u in si.on_update:
                    if u.update_mode != "sem-inc":
                        nonmono.add(u.ant_name)

    def implied(k, sem, val):
        return k.get(sem, 0) >= val

    def learn(k, sem, val):
        if k.get(sem, 0) < val:
            k[sem] = val
        # transitively absorb the producer's snapshot at this update
        hist = sem_hist.get(sem)
        if hist:
            import bisect
            i = bisect.bisect_left([h[0] for h in hist], val)
            if i < len(hist):
                for s2, v2 in hist[i][1].items():
                    if k.get(s2, 0) < v2:
                        k[s2] = v2

    from concourse import mybir

    for b in nc.m.functions[0].blocks:
        new_instructions = []
        for inst in b.instructions:
            si = inst.sync_info
            eng = str(inst.engine)
            k = know.setdefault(eng, {})
            if si is not None:
                waits = list(si.on_wait)
                if type(inst).__name__ == "InstDrain" and len(waits) > 1:
                    dma = [w for w in waits if "DMA" in w.ant_name]
                    keep_w = dma[-1:] if dma else waits[:1]
                    for w in waits:
                        learn(k, w.ant_name, w.wait_value)
                else:
                    merged = {}
                    for w in waits:
                        if w.ant_name in nonmono:
                            merged[id(w)] = w
                        elif w.ant_name not in merged or \
                                merged[w.ant_name].wait_value < w.wait_value:
                            merged[w.ant_name] = w
                    keep_w = []
                    for w in merged.values():
                        if w.ant_name in nonmono:
                            keep_w.append(w)
                            continue
                        if not implied(k, w.ant_name, w.wait_value):
                            keep_w.append(w)
                        learn(k, w.ant_name, w.wait_value)
                    # walrus has one wait slot per instruction: hoist extra
                    # waits onto injected no-ops on the same engine
                    while len(keep_w) > 1:
                        w = keep_w.pop(0)
                        nop = mybir.InstNoOp(
                            name=f"{inst.name}_w{len(keep_w)}",
                            engine=inst.engine, ins=[], outs=[],
                            sync_info=bass_rust.SyncInfo(
                                on_wait=[w], on_update=[]))
                        new_instructions.append(nop)
                if len(keep_w) != len(waits):
                    inst.sync_info = bass_rust.SyncInfo(
                        on_wait=keep_w, on_update=list(si.on_update))
                    si = inst.sync_info
                for u in si.on_update:
                    if u.update_mode == "sem-inc" and u.ant_name not in nonmono:
                        name = u.ant_name
                        cum = sem_cum.get(name, 0) + u.update_value
                        sem_cum[name] = cum
                        # own-engine sems are implicitly ordered for later
                        # instructions on the same queue
                        pref = eng_sem.get(eng)
                        if pref and name.startswith(pref):
                            k[name] = max(k.get(name, 0), cum)
                        sem_hist.setdefault(name, []).append((cum, dict(k)))
            new_instructions.append(inst)
        if len(new_instructions) != len(b.instructions):
            b.instructions = new_instructions


def kernel(llr, max_iters):
    llr = np.ascontiguousarray(np.asarray(llr), dtype=np.float32)
    iters = int(np.asarray(max_iters))
    B = llr.shape[0]
    if iters <= 0:
        return llr.reshape(B, 1, 7).copy()

    from concourse.bass_utils import run_bass_kernel_spmd

    Bc = B // NCORES
    key = (Bc, iters)
    if key not in _CACHE:
        _CACHE[key] = _build(Bc, iters)
    nc = _CACHE[key]

    flat = llr.reshape(B, 7)
    in_maps = [{"llr": flat[i * Bc:(i + 1) * Bc]} for i in range(NCORES)]
    res = run_bass_kernel_spmd(nc, in_maps, core_ids=list(range(NCORES)))
    out = np.concatenate([np.asarray(r["out"]) for r in res.results], axis=0)
    return out.reshape(B, 1, 7)


# revision 52
# speedup vs baseline: 1.0148x; 1.0003x over previous
"""LDPC belief-propagation kernel for Trainium2 (8 NeuronCores, data-parallel).

Math (per batch row, H fixed [3,7], 12 edges):
  t_e   = tanh(m_e / 2)                       (signed!)
  u_e   = prod_{e' in check c, e' != e} t_e'  (signed leave-one-out product)
  c2v_e = 2 atanh(u_e) = ln(1+u) - ln(1-u)    (signed, sign handled for free)
  new_llr_v = llr_v + sum_{c contains v} c2v_{c,v}
  m'_e  = new_llr_v - c2v_e
Only Tanh/Ln tables; the sign pipeline of the classic phi/phi formulation
disappears because the tanh products carry signs natively.

Edge layout is role-major per 12-slot group: [s0 s1 s2 | a0 a1 a2 | b0 b1 b2 |
d0 d1 d2] where s_c is the check's degree-1-variable edge (v0,v1,v3 - their
messages never change), d_c is v6's edge in check c, and (a, b) =
((v2c0, v2c1, v4c2), (v4c0, v5c1, v5c2)).  This makes every structural op a
single strided instruction:
  Q[k]   = T[k] * T[k+6]         (k=0..5: pair products (s*b, a*d) per check)
  U[3..8]  = T[(9,10,11,0,1,2)] * Q[0..5]   (loo for roles a, b)
  U[9..11] = T[3..5] * Q[0..2]              (loo for role d)
  M'[deg2 six edges] = LB6 + CV[partner]    (partner = pair-swap view)
  M'[d]  = (c2v_d-sum + llr_6) - CV[d]      (v6 leave-one-out via total sum)
Batch is split into 3 width-tuned chunks (87/91/78 columns per partition)
whose iterations run staggered, so ACT (Tanh + the two Ln ops), DVE
(products, c2v subtract, message updates) and Pool (d-role products, v6
pair-sums) stay concurrently busy; iteration 0 reads tanh(llr) directly so
the edge-slot scatter stays off the pipeline-fill critical path, and
setup copies / output DMAs are placed by schedule hints to keep the fill
and drain edges tight.  Sync info is reduced to walrus's one-wait-slot
limit by a vector-clock pass (_strip_syncs).
"""

import numpy as np

_CACHE = {}

NCORES = 8
P = 128      # partitions
CHUNKS = 3   # batch sub-chunks per core (pipeline depth)

# guard so ln(1 -+ 0.99999988*u) stays finite (>= ~1.2e-7) even at u = -+1
LNSCALE = 0.99999988

# (start, half-period, partA->partB gap) ns hints for the list scheduler
_SCHED = (0, 0, 0)

# schedule hints deferring chunk setup copies (ns)
_CPY = (0, 9800, 12300)

# manual chunk widths (must sum to Bc//P//1); None = near-even
_WS = (87, 91, 78)


def _build(Bc, iters):
    import contextlib

    import concourse.bass as bass
    import concourse.tile as tile
    from concourse import mybir
    from concourse.alu_op_type import AluOpType as Op

    F = mybir.ActivationFunctionType
    Wtot = Bc // P
    if _WS is not None and sum(_WS) == Wtot:
        Ws = list(_WS)
    else:
        base, rem = divmod(Wtot, CHUNKS)
        Ws = [base + (1 if i == 1 else 0) for i in range(CHUNKS)] \
            if rem == 1 else \
            [base + (1 if i >= CHUNKS - rem else 0) for i in range(CHUNKS)]
    f32 = mybir.dt.float32

    f16 = mybir.dt.float16
    nc = bass.Bass("TRN2", target_bir_lowering=False, debug=False,
                   num_devices=1)
    llr_d = nc.dram_tensor("llr", [Bc, 7], f32, kind="ExternalInput")
    out_d = nc.dram_tensor("out", [Bc, 7], f32, kind="ExternalOutput")

    def sub(t, off, dims):
        a = t[:] if callable(getattr(t, "__getitem__", None)) else t
        return bass.AP(tensor=a.tensor, offset=a.offset + off,
                       ap=[list(a.ap[0])] + [list(d) for d in dims])

    with tile.TileContext(nc) as tc:
        ctx = contextlib.ExitStack()
        with ctx:
            keep = ctx.enter_context(tc.tile_pool(name="keep", bufs=1))
            work = ctx.enter_context(tc.tile_pool(name="work", bufs=2))

            def K(name, c, k, dt=f32):
                return keep.tile([P, Ws[c] * k], dt, tag=name, name=name)

            # per-chunk persistent state
            LLs = [K(f"LL{c}", c, 7) for c in range(CHUNKS)]    # llr, natural v order
            LBs = [K(f"LB{c}", c, 6, f16) for c in range(CHUNKS)]   # llr bcast, deg2 edges
            L6s = [K(f"L6{c}", c, 3, f16) for c in range(CHUNKS)]   # llr6 bcast, v6 edges
            Ts  = [K(f"T{c}", c, 12) for c in range(CHUNKS)]    # tanh(m/2) per edge
            Ms  = [K(f"M{c}", c, 9, f16) for c in range(CHUNKS)]    # dyn messages
            NLs = [K(f"NL{c}", c, 7) for c in range(CHUNKS)]    # output llr

            act = nc.scalar.activation
            vec = nc.vector
            gps = nc.gpsimd

            def dram_view(t, c, w0, nw):
                # [P, nw*7] window of chunk c: rows base_c + p*Ws[c] + w
                a = t.ap()
                off = (P * sum(Ws[:c]) + w0) * 7
                return bass.AP(tensor=a.tensor, offset=a.offset + off,
                               ap=[[Ws[c] * 7, P], [1, nw * 7]])

            for c in range(CHUNKS):
                eng = nc.sync if c == 0 else nc.gpsimd
                eng.dma_start(out=LLs[c][:], in_=dram_view(llr_d, c, 0, Ws[c]))

            cur = {"W": Ws[0]}

            def v7(t, off, *dims):
                return sub(t, off, [[7, cur["W"]]] + [list(d) for d in dims])

            def v12(t, off, *dims):
                return sub(t, off, [[12, cur["W"]]] + [list(d) for d in dims])

            def v9(t, off, *dims):
                return sub(t, off, [[9, cur["W"]]] + [list(d) for d in dims])

            def v6(t, off, *dims):
                return sub(t, off, [[6, cur["W"]]] + [list(d) for d in dims])

            def v3(t, off, *dims):
                return sub(t, off, [[3, cur["W"]]] + [list(d) for d in dims])

            state = [{} for _ in range(CHUNKS)]

            def partA(c, it):
                """tanh + products: T, Q, U."""
                LL, LB, T, M = LLs[c], LBs[c], Ts[c], Ms[c]
                last = (it == iters - 1)
                W = Ws[c]
                cur["W"] = W
                Q = work.tile([P, W * 6], f32, tag=f"Q{c}", name="Q")
                U = work.tile([P, W * 12], f32, tag=f"U{c}", name="U")
                state[c] = {"Q": Q, "U": U}

                if it == 0:
                    # t = tanh(llr/2) once; iteration-0 products read TL
                    # directly so the T scatter stays off the critical path
                    TL = work.tile([P, W * 7], f32, tag=f"TL{c}", name="TL")
                    act(TL[:], LL[:], F.Tanh, scale=0.5)
                    vec.tensor_tensor(v6(Q, 0, [1, 2]), v7(TL, 0, [1, 2]),
                                      v7(TL, 4, [1, 2]), Op.mult)
                    vec.tensor_tensor(v6(Q, 2, [1, 1]), v7(TL, 3, [1, 1]),
                                      v7(TL, 5, [1, 1]), Op.mult)
                    gps.tensor_tensor(v6(Q, 3, [1, 2]), v7(TL, 2, [0, 2]),
                                      v7(TL, 6, [0, 2]), Op.mult)
                    gps.tensor_tensor(v6(Q, 5, [1, 1]), v7(TL, 4, [1, 1]),
                                      v7(TL, 6, [1, 1]), Op.mult)
                    vec.tensor_tensor(v12(U, 3, [1, 3]), v7(TL, 6, [0, 3]),
                                      v6(Q, 0, [1, 3]), Op.mult)
                    vec.tensor_tensor(v12(U, 6, [1, 2]), v7(TL, 0, [1, 2]),
                                      v6(Q, 3, [1, 2]), Op.mult)
                    vec.tensor_tensor(v12(U, 8, [1, 1]), v7(TL, 3, [1, 1]),
                                      v6(Q, 5, [1, 1]), Op.mult)
                    gps.tensor_tensor(v12(U, 9, [1, 2]), v7(TL, 2, [0, 2]),
                                      v6(Q, 0, [1, 2]), Op.mult)
                    gps.tensor_tensor(v12(U, 11, [1, 1]), v7(TL, 4, [1, 1]),
                                      v6(Q, 2, [1, 1]), Op.mult)
                    if last:  # iters == 1
                        vec.tensor_tensor(v12(U, 0, [1, 2]), v7(TL, 4, [1, 2]),
                                          v6(Q, 3, [1, 2]), Op.mult)
                        vec.tensor_tensor(v12(U, 2, [1, 1]), v7(TL, 5, [1, 1]),
                                          v6(Q, 5, [1, 1]), Op.mult)
                    # scatter t to role-major slots for later iterations,
                    # off the critical path (only statics strictly needed
                    # before iteration 1's products)
                    vec.tensor_copy(v12(T, 0, [1, 2]), v7(TL, 0, [1, 2]))
                    vec.tensor_copy(v12(T, 2, [1, 1]), v7(TL, 3, [1, 1]))
                    with tc.tile_wait_until(_CPY[c] / 1e6, enable=_CPY[c] > 0):
                        gps.tensor_copy(v6(LB, 0, [1, 2]), v7(LL, 2, [0, 2]))
                        gps.tensor_copy(v6(LB, 2, [1, 4]),
                                        v7(LL, 4, [1, 2], [0, 2]))
                        gps.tensor_copy(v3(L6s[c], 0, [1, 3]),
                                        v7(LL, 6, [0, 3]))
                else:
                    act(v12(T, 3, [1, 9]), M[:], F.Tanh, scale=0.5)
                    # pair products and signed leave-one-out products
                    vec.tensor_tensor(Q[:], v12(T, 0, [1, 6]),
                                      v12(T, 6, [1, 6]), Op.mult)
                    vec.tensor_tensor(v12(U, 3, [1, 6]),
                                      v12(T, 9, [-9, 2], [1, 3]),
                                      v6(Q, 0, [1, 6]), Op.mult)
                    gps.tensor_tensor(v12(U, 9, [1, 3]), v12(T, 3, [1, 3]),
                                      v6(Q, 0, [1, 3]), Op.mult)
                    if last:
                        vec.tensor_tensor(v12(U, 0, [1, 3]), v12(T, 6, [1, 3]),
                                          v6(Q, 3, [1, 3]), Op.mult)

            def partB(c, it):
                """c2v + message/new-llr update."""
                LL, LB, M, NL = LLs[c], LBs[c], Ms[c], NLs[c]
                last = (it == iters - 1)
                W = Ws[c]
                cur["W"] = W
                U = state[c]["U"]
                LP = work.tile([P, W * 12], f16, tag=f"LP{c}", name="LP")
                LM = work.tile([P, W * 12], f16, tag=f"LM{c}", name="LM")
                CV = work.tile([P, W * 12], f16, tag=f"CV{c}", name="CV")

                off, n = (0, 9) if last else (3, 6)
                # c2v = ln(1+u) - ln(1-u), guarded away from ln(0)
                act(v12(LP, off, [1, n + 3]), v12(U, off, [1, n + 3]), F.Ln,
                    bias=1.0, scale=LNSCALE)
                act(v12(LM, off, [1, n + 3]), v12(U, off, [1, n + 3]), F.Ln,
                    bias=1.0, scale=-LNSCALE)
                vec.tensor_tensor(v12(CV, off, [1, n + 3]),
                                  v12(LP, off, [1, n + 3]),
                                  v12(LM, off, [1, n + 3]), Op.subtract)

                # v6 leave-one-out sums of the d-role c2vs, depth 2:
                # X[0]=c10+c11  X[1]=c9+c11  X[2]=c9+c10
                X = work.tile([P, W * 3], f16, tag=f"X{c}", name="X")
                if not last:
                    # X[0], X[1] feed only the mid-iteration v6 m'-update
                    vec.tensor_tensor(v3(X, 0, [1, 2]), v12(CV, 10, [-1, 2]),
                                      v12(CV, 11, [0, 2]), Op.add)
                gps.tensor_tensor(v3(X, 2, [1, 1]), v12(CV, 9, [1, 1]),
                                  v12(CV, 10, [1, 1]), Op.add)

                if not last:
                    # m' for the six deg-2 edges: llr + partner c2v
                    vec.tensor_tensor(v9(M, 0, [1, 6]), v6(LB, 0, [1, 6]),
                                      v12(CV, 4, [2, 3], [-1, 2]), Op.add)
                    # m' for v6 edges: llr6 + sum of the other two c2v_d
                    vec.tensor_tensor(v9(M, 6, [1, 3]), v3(L6s[c], 0, [1, 3]),
                                      v3(X, 0, [1, 3]), Op.add)
                else:
                    # new_llr in natural variable order
                    SP = work.tile([P, W * 3], f32, tag=f"SP{c}", name="SP")
                    gps.tensor_tensor(v7(NL, 0, [1, 2]), v7(LL, 0, [1, 2]),
                                      v12(CV, 0, [1, 2]), Op.add)
                    gps.tensor_tensor(v7(NL, 3, [1, 1]), v7(LL, 3, [1, 1]),
                                      v12(CV, 2, [1, 1]), Op.add)
                    vec.tensor_tensor(SP[:], v12(CV, 3, [2, 3]),
                                      v12(CV, 4, [2, 3]), Op.add)
                    vec.tensor_tensor(v7(NL, 2, [1, 1]), v7(LL, 2, [1, 1]),
                                      v3(SP, 0, [1, 1]), Op.add)
                    vec.tensor_tensor(v7(NL, 4, [1, 2]), v7(LL, 4, [1, 2]),
                                      v3(SP, 1, [1, 2]), Op.add)
                    S1 = work.tile([P, W], f32, tag=f"S1{c}", name="S1")
                    vec.tensor_tensor(S1[:], v12(CV, 11, [1, 1]),
                                      v7(LL, 6, [1, 1]), Op.add)
                    vec.tensor_tensor(v7(NL, 6, [1, 1]), v3(X, 2, [1, 1]),
                                      S1[:], Op.add)
                    wl = W // 2
                    wh = W - wl
                    lo = bass.AP(tensor=NL[:].tensor, offset=NL[:].offset,
                                 ap=[list(NL[:].ap[0])] + [[7, wl], [1, 7]])
                    hi = bass.AP(tensor=NL[:].tensor,
                                 offset=NL[:].offset + wl * 7,
                                 ap=[list(NL[:].ap[0])] + [[7, wh], [1, 7]])
                    e0, e1 = ((nc.sync, nc.sync) if c < CHUNKS - 1
                              else (nc.sync, nc.gpsimd))
                    e0.dma_start(out=dram_view(out_d, c, 0, wl), in_=lo)
                    e1.dma_start(out=dram_view(out_d, c, wl, wh), in_=hi)

            # software-pipelined schedule: chunk 1 runs half an iteration
            # behind chunk 0 so each chunk's ACT phase (Tanh / Ln Ln) overlaps
            # the other chunk's vector phase (products / updates).  The
            # wait-until timestamps steer the Tile list scheduler into that
            # stagger; they are lower bounds only, data deps still rule.
            S0, HALF, GAP = _SCHED
            for it in range(iters):
                for c in range(CHUNKS):
                    tA = S0 + (CHUNKS * it + c) * HALF
                    with tc.tile_wait_until(tA / 1e6, enable=tA > 0):
                        partA(c, it)
                    with tc.tile_wait_until((tA + GAP) / 1e6):
                        partB(c, it)

    _strip_syncs(nc)
    return nc


def _strip_syncs(nc):
    """walrus on this stack supports a single sync-wait slot per instruction.
    Reduce each instruction's wait list via a vector-clock pass: walking the
    scheduled program order, every engine accumulates knowledge of semaphore
    values - from its own queue position, from waits it has already performed,
    and transitively from the producer's knowledge snapshot at the awaited
    update.  A wait already implied by that knowledge is dropped.  Kernel-tail
    drains keep only their DMA wait (the per-engine drain + EVSEM butterfly
    that follows enforces engine completion)."""
    import bass_rust

    eng_sem = {"EngineType.DVE": "DVE_", "EngineType.Pool": "Pool_",
               "EngineType.Activation": "Activation_", "EngineType.PE": "PE_",
               "EngineType.SP": "SP_"}
    know = {e: {} for e in eng_sem}          # engine -> {sem: value}
    sem_hist = {}                            # sem -> list of (cum_value, snapshot)
    sem_cum = {}                             # sem -> cumulative inc so far

    # Sems that are ever decremented (barrier gather sems) are not monotone;
    # leave their waits untouched and keep them out of the knowledge model.
    nonmono = set()
    for b in nc.m.functions[0].blocks:
        for inst in b.instructions:
            si = inst.sync_info
            if si is not None:
                for u in si.on_update:
                    if u.update_mode != "sem-inc":
                        nonmono.add(u.ant_name)

    def implied(k, sem, val):
        return k.get(sem, 0) >= val

    def learn(k, sem, val):
        if k.get(sem, 0) < val:
            k[sem] = val
        # transitively absorb the producer's snapshot at this update
        hist = sem_hist.get(sem)
        if hist:
            import bisect
            i = bisect.bisect_left([h[0] for h in hist], val)
            if i < len(hist):
                for s2, v2 in hist[i][1].items():
                    if k.get(s2, 0) < v2:
                        k[s2] = v2

    from concourse import mybir

    for b in nc.m.functions[0].blocks:
        new_instructions = []
        for inst in b.instructions:
            si = inst.sync_info
            eng = str(inst.engine)
            k = know.setdefault(eng, {})
            if si is not None:
                waits = list(si.on_wait)
                if type(inst).__name__ == "InstDrain" and len(waits) > 1:
                    dma = [w for w in waits if "DMA" in w.ant_name]
                    keep_w = dma[-1:] if dma else waits[:1]
                    for w in waits:
                        learn(k, w.ant_name, w.wait_value)
                else:
                    merged = {}
                    for w in waits:
                        if w.ant_name in nonmono:
                            merged[id(w)] = w
                        elif w.ant_name not in merged or \
                                merged[w.ant_name].wait_value < w.wait_value:
                            merged[w.ant_name] = w
                    keep_w = []
                    for w in merged.values():
                        if w.ant_name in nonmono:
                            keep_w.append(w)
                            continue
                        if not implied(k, w.ant_name, w.wait_value):
                            keep_w.append(w)
                        learn(k, w.ant_name, w.wait_value)
                    # walrus has one wait slot per instruction: hoist extra
                    # waits onto injected no-ops on the same engine
                    while len(keep_w) > 1:
                        w = keep_w.pop(0)
                        nop = mybir.InstNoOp(
                            name=f"{inst.name}_w{len(keep_w)}",
                            engine=inst.engine, ins=[], outs=[],
                            sync_info=bass_rust.SyncInfo(
                                on_wait=[w], on_update=[]))
                        new_instructions.append(nop)
                if len(keep_w) != len(waits):
                    inst.sync_info = bass_rust.SyncInfo(
                        on_wait=keep_w, on_update=list(si.on_update))
                    si = inst.sync_info
                for u in si.on_update:
                    if u.update_mode == "sem-inc" and u.ant_name not in nonmono:
                        name = u.ant_name
                        cum = sem_cum.get(name, 0) + u.update_value
                        sem_cum[name] = cum
                        # own-engine sems are implicitly ordered for later
                        # instructions on the same queue
                        pref = eng_sem.get(eng)
                        if pref and name.startswith(pref):
                            k[name] = max(k.get(name, 0), cum)
                        sem_hist.setdefault(name, []).append((cum, dict(k)))
            new_instructions.append(inst)
        if len(new_instructions) != len(b.instructions):
            b.instructions = new_instructions


def kernel(llr, max_iters):
    llr = np.ascontiguousarray(np.asarray(llr), dtype=np.float32)
    iters = int(np.asarray(max_iters))
    B = llr.shape[0]
    if iters <= 0:
        return llr.reshape(B, 1, 7).copy()

    from concourse.bass_utils import run_bass_kernel_spmd

    Bc = B // NCORES
    key = (Bc, iters)
    if key not in _CACHE:
        _CACHE[key] = _build(Bc, iters)
    nc = _CACHE[key]

    flat = llr.reshape(B, 7)
    in_maps = [{"llr": flat[i * Bc:(i + 1) * Bc]} for i in range(NCORES)]
    res = run_bass_kernel_spmd(nc, in_maps, core_ids=list(range(NCORES)))
    out = np.concatenate([np.asarray(r["out"]) for r in res.results], axis=0)
    return out.reshape(B, 1, 7)


# revision 53
# speedup vs baseline: 1.0159x; 1.0011x over previous
"""LDPC belief-propagation kernel for Trainium2 (8 NeuronCores, data-parallel).

Math (per batch row, H fixed [3,7], 12 edges):
  t_e   = tanh(m_e / 2)                       (signed!)
  u_e   = prod_{e' in check c, e' != e} t_e'  (signed leave-one-out product)
  c2v_e = 2 atanh(u_e) = ln(1+u) - ln(1-u)    (signed, sign handled for free)
  new_llr_v = llr_v + sum_{c contains v} c2v_{c,v}
  m'_e  = new_llr_v - c2v_e
Only Tanh/Ln tables; the sign pipeline of the classic phi/phi formulation
disappears because the tanh products carry signs natively.

Edge layout is role-major per 12-slot group: [s0 s1 s2 | a0 a1 a2 | b0 b1 b2 |
d0 d1 d2] where s_c is the check's degree-1-variable edge (v0,v1,v3 - their
messages never change), d_c is v6's edge in check c, and (a, b) =
((v2c0, v2c1, v4c2), (v4c0, v5c1, v5c2)).  This makes every structural op a
single strided instruction:
  Q[k]   = T[k] * T[k+6]         (k=0..5: pair products (s*b, a*d) per check)
  U[3..8]  = T[(9,10,11,0,1,2)] * Q[0..5]   (loo for roles a, b)
  U[9..11] = T[3..5] * Q[0..2]              (loo for role d)
  M'[deg2 six edges] = LB6 + CV[partner]    (partner = pair-swap view)
  M'[d]  = (c2v_d-sum + llr_6) - CV[d]      (v6 leave-one-out via total sum)
Batch is split into 3 width-tuned chunks (87/92/77 columns per partition)
whose iterations run staggered, so ACT (Tanh + the two Ln ops), DVE
(products, c2v subtract, message updates) and Pool (d-role products, v6
pair-sums) stay concurrently busy; iteration 0 reads tanh(llr) directly so
the edge-slot scatter stays off the pipeline-fill critical path, and
setup copies / output DMAs are placed by schedule hints to keep the fill
and drain edges tight.  Sync info is reduced to walrus's one-wait-slot
limit by a vector-clock pass (_strip_syncs).
"""

import numpy as np

_CACHE = {}

NCORES = 8
P = 128      # partitions
CHUNKS = 3   # batch sub-chunks per core (pipeline depth)

# guard so ln(1 -+ 0.99999988*u) stays finite (>= ~1.2e-7) even at u = -+1
LNSCALE = 0.99999988

# (start, half-period, partA->partB gap) ns hints for the list scheduler
_SCHED = (0, 0, 0)

# schedule hints deferring chunk setup copies (ns)
_CPY = (0, 9800, 12300)

# manual chunk widths (must sum to Bc//P//1); None = near-even
_WS = (87, 92, 77)


def _build(Bc, iters):
    import contextlib

    import concourse.bass as bass
    import concourse.tile as tile
    from concourse import mybir
    from concourse.alu_op_type import AluOpType as Op

    F = mybir.ActivationFunctionType
    Wtot = Bc // P
    if _WS is not None and sum(_WS) == Wtot:
        Ws = list(_WS)
    else:
        base, rem = divmod(Wtot, CHUNKS)
        Ws = [base + (1 if i == 1 else 0) for i in range(CHUNKS)] \
            if rem == 1 else \
            [base + (1 if i >= CHUNKS - rem else 0) for i in range(CHUNKS)]
    f32 = mybir.dt.float32

    f16 = mybir.dt.float16
    nc = bass.Bass("TRN2", target_bir_lowering=False, debug=False,
                   num_devices=1)
    llr_d = nc.dram_tensor("llr", [Bc, 7], f32, kind="ExternalInput")
    out_d = nc.dram_tensor("out", [Bc, 7], f32, kind="ExternalOutput")

    def sub(t, off, dims):
        a = t[:] if callable(getattr(t, "__getitem__", None)) else t
        return bass.AP(tensor=a.tensor, offset=a.offset + off,
                       ap=[list(a.ap[0])] + [list(d) for d in dims])

    with tile.TileContext(nc) as tc:
        ctx = contextlib.ExitStack()
        with ctx:
            keep = ctx.enter_context(tc.tile_pool(name="keep", bufs=1))
            work = ctx.enter_context(tc.tile_pool(name="work", bufs=2))

            def K(name, c, k, dt=f32):
                return keep.tile([P, Ws[c] * k], dt, tag=name, name=name)

            # per-chunk persistent state
            LLs = [K(f"LL{c}", c, 7) for c in range(CHUNKS)]    # llr, natural v order
            LBs = [K(f"LB{c}", c, 6, f16) for c in range(CHUNKS)]   # llr bcast, deg2 edges
            L6s = [K(f"L6{c}", c, 3, f16) for c in range(CHUNKS)]   # llr6 bcast, v6 edges
            Ts  = [K(f"T{c}", c, 12) for c in range(CHUNKS)]    # tanh(m/2) per edge
            Ms  = [K(f"M{c}", c, 9, f16) for c in range(CHUNKS)]    # dyn messages
            NLs = [K(f"NL{c}", c, 7) for c in range(CHUNKS)]    # output llr

            act = nc.scalar.activation
            vec = nc.vector
            gps = nc.gpsimd

            def dram_view(t, c, w0, nw):
                # [P, nw*7] window of chunk c: rows base_c + p*Ws[c] + w
                a = t.ap()
                off = (P * sum(Ws[:c]) + w0) * 7
                return bass.AP(tensor=a.tensor, offset=a.offset + off,
                               ap=[[Ws[c] * 7, P], [1, nw * 7]])

            for c in range(CHUNKS):
                eng = nc.sync if c == 0 else nc.gpsimd
                eng.dma_start(out=LLs[c][:], in_=dram_view(llr_d, c, 0, Ws[c]))

            cur = {"W": Ws[0]}

            def v7(t, off, *dims):
                return sub(t, off, [[7, cur["W"]]] + [list(d) for d in dims])

            def v12(t, off, *dims):
                return sub(t, off, [[12, cur["W"]]] + [list(d) for d in dims])

            def v9(t, off, *dims):
                return sub(t, off, [[9, cur["W"]]] + [list(d) for d in dims])

            def v6(t, off, *dims):
                return sub(t, off, [[6, cur["W"]]] + [list(d) for d in dims])

            def v3(t, off, *dims):
                return sub(t, off, [[3, cur["W"]]] + [list(d) for d in dims])

            state = [{} for _ in range(CHUNKS)]

            def partA(c, it):
                """tanh + products: T, Q, U."""
                LL, LB, T, M = LLs[c], LBs[c], Ts[c], Ms[c]
                last = (it == iters - 1)
                W = Ws[c]
                cur["W"] = W
                Q = work.tile([P, W * 6], f32, tag=f"Q{c}", name="Q")
                U = work.tile([P, W * 12], f32, tag=f"U{c}", name="U")
                state[c] = {"Q": Q, "U": U}

                if it == 0:
                    # t = tanh(llr/2) once; iteration-0 products read TL
                    # directly so the T scatter stays off the critical path
                    TL = work.tile([P, W * 7], f32, tag=f"TL{c}", name="TL")
                    act(TL[:], LL[:], F.Tanh, scale=0.5)
                    vec.tensor_tensor(v6(Q, 0, [1, 2]), v7(TL, 0, [1, 2]),
                                      v7(TL, 4, [1, 2]), Op.mult)
                    vec.tensor_tensor(v6(Q, 2, [1, 1]), v7(TL, 3, [1, 1]),
                                      v7(TL, 5, [1, 1]), Op.mult)
                    gps.tensor_tensor(v6(Q, 3, [1, 2]), v7(TL, 2, [0, 2]),
                                      v7(TL, 6, [0, 2]), Op.mult)
                    gps.tensor_tensor(v6(Q, 5, [1, 1]), v7(TL, 4, [1, 1]),
                                      v7(TL, 6, [1, 1]), Op.mult)
                    vec.tensor_tensor(v12(U, 3, [1, 3]), v7(TL, 6, [0, 3]),
                                      v6(Q, 0, [1, 3]), Op.mult)
                    vec.tensor_tensor(v12(U, 6, [1, 2]), v7(TL, 0, [1, 2]),
                                      v6(Q, 3, [1, 2]), Op.mult)
                    vec.tensor_tensor(v12(U, 8, [1, 1]), v7(TL, 3, [1, 1]),
                                      v6(Q, 5, [1, 1]), Op.mult)
                    gps.tensor_tensor(v12(U, 9, [1, 2]), v7(TL, 2, [0, 2]),
                                      v6(Q, 0, [1, 2]), Op.mult)
                    gps.tensor_tensor(v12(U, 11, [1, 1]), v7(TL, 4, [1, 1]),
                                      v6(Q, 2, [1, 1]), Op.mult)
                    if last:  # iters == 1
                        vec.tensor_tensor(v12(U, 0, [1, 2]), v7(TL, 4, [1, 2]),
                                          v6(Q, 3, [1, 2]), Op.mult)
                        vec.tensor_tensor(v12(U, 2, [1, 1]), v7(TL, 5, [1, 1]),
                                          v6(Q, 5, [1, 1]), Op.mult)
                    # scatter t to role-major slots for later iterations,
                    # off the critical path (only statics strictly needed
                    # before iteration 1's products)
                    vec.tensor_copy(v12(T, 0, [1, 2]), v7(TL, 0, [1, 2]))
                    vec.tensor_copy(v12(T, 2, [1, 1]), v7(TL, 3, [1, 1]))
                    with tc.tile_wait_until(_CPY[c] / 1e6, enable=_CPY[c] > 0):
                        gps.tensor_copy(v6(LB, 0, [1, 2]), v7(LL, 2, [0, 2]))
                        gps.tensor_copy(v6(LB, 2, [1, 4]),
                                        v7(LL, 4, [1, 2], [0, 2]))
                        gps.tensor_copy(v3(L6s[c], 0, [1, 3]),
                                        v7(LL, 6, [0, 3]))
                else:
                    act(v12(T, 3, [1, 9]), M[:], F.Tanh, scale=0.5)
                    # pair products and signed leave-one-out products
                    vec.tensor_tensor(Q[:], v12(T, 0, [1, 6]),
                                      v12(T, 6, [1, 6]), Op.mult)
                    vec.tensor_tensor(v12(U, 3, [1, 6]),
                                      v12(T, 9, [-9, 2], [1, 3]),
                                      v6(Q, 0, [1, 6]), Op.mult)
                    gps.tensor_tensor(v12(U, 9, [1, 3]), v12(T, 3, [1, 3]),
                                      v6(Q, 0, [1, 3]), Op.mult)
                    if last:
                        vec.tensor_tensor(v12(U, 0, [1, 3]), v12(T, 6, [1, 3]),
                                          v6(Q, 3, [1, 3]), Op.mult)

            def partB(c, it):
                """c2v + message/new-llr update."""
                LL, LB, M, NL = LLs[c], LBs[c], Ms[c], NLs[c]
                last = (it == iters - 1)
                W = Ws[c]
                cur["W"] = W
                U = state[c]["U"]
                LP = work.tile([P, W * 12], f16, tag=f"LP{c}", name="LP")
                LM = work.tile([P, W * 12], f16, tag=f"LM{c}", name="LM")
                CV = work.tile([P, W * 12], f16, tag=f"CV{c}", name="CV")

                off, n = (0, 9) if last else (3, 6)
                # c2v = ln(1+u) - ln(1-u), guarded away from ln(0)
                act(v12(LP, off, [1, n + 3]), v12(U, off, [1, n + 3]), F.Ln,
                    bias=1.0, scale=LNSCALE)
                act(v12(LM, off, [1, n + 3]), v12(U, off, [1, n + 3]), F.Ln,
                    bias=1.0, scale=-LNSCALE)
                vec.tensor_tensor(v12(CV, off, [1, n + 3]),
                                  v12(LP, off, [1, n + 3]),
                                  v12(LM, off, [1, n + 3]), Op.subtract)

                # v6 leave-one-out sums of the d-role c2vs, depth 2:
                # X[0]=c10+c11  X[1]=c9+c11  X[2]=c9+c10
                X = work.tile([P, W * 3], f16, tag=f"X{c}", name="X")
                if not last:
                    # X[0], X[1] feed only the mid-iteration v6 m'-update
                    vec.tensor_tensor(v3(X, 0, [1, 2]), v12(CV, 10, [-1, 2]),
                                      v12(CV, 11, [0, 2]), Op.add)
                gps.tensor_tensor(v3(X, 2, [1, 1]), v12(CV, 9, [1, 1]),
                                  v12(CV, 10, [1, 1]), Op.add)

                if not last:
                    # m' for the six deg-2 edges: llr + partner c2v
                    vec.tensor_tensor(v9(M, 0, [1, 6]), v6(LB, 0, [1, 6]),
                                      v12(CV, 4, [2, 3], [-1, 2]), Op.add)
                    # m' for v6 edges: llr6 + sum of the other two c2v_d
                    vec.tensor_tensor(v9(M, 6, [1, 3]), v3(L6s[c], 0, [1, 3]),
                                      v3(X, 0, [1, 3]), Op.add)
                else:
                    # new_llr in natural variable order
                    SP = work.tile([P, W * 3], f32, tag=f"SP{c}", name="SP")
                    gps.tensor_tensor(v7(NL, 0, [1, 2]), v7(LL, 0, [1, 2]),
                                      v12(CV, 0, [1, 2]), Op.add)
                    gps.tensor_tensor(v7(NL, 3, [1, 1]), v7(LL, 3, [1, 1]),
                                      v12(CV, 2, [1, 1]), Op.add)
                    vec.tensor_tensor(SP[:], v12(CV, 3, [2, 3]),
                                      v12(CV, 4, [2, 3]), Op.add)
                    vec.tensor_tensor(v7(NL, 2, [1, 1]), v7(LL, 2, [1, 1]),
                                      v3(SP, 0, [1, 1]), Op.add)
                    vec.tensor_tensor(v7(NL, 4, [1, 2]), v7(LL, 4, [1, 2]),
                                      v3(SP, 1, [1, 2]), Op.add)
                    S1 = work.tile([P, W], f32, tag=f"S1{c}", name="S1")
                    vec.tensor_tensor(S1[:], v12(CV, 11, [1, 1]),
                                      v7(LL, 6, [1, 1]), Op.add)
                    vec.tensor_tensor(v7(NL, 6, [1, 1]), v3(X, 2, [1, 1]),
                                      S1[:], Op.add)
                    wl = W // 2
                    wh = W - wl
                    lo = bass.AP(tensor=NL[:].tensor, offset=NL[:].offset,
                                 ap=[list(NL[:].ap[0])] + [[7, wl], [1, 7]])
                    hi = bass.AP(tensor=NL[:].tensor,
                                 offset=NL[:].offset + wl * 7,
                                 ap=[list(NL[:].ap[0])] + [[7, wh], [1, 7]])
                    e0, e1 = ((nc.sync, nc.sync) if c < CHUNKS - 1
                              else (nc.sync, nc.gpsimd))
                    e0.dma_start(out=dram_view(out_d, c, 0, wl), in_=lo)
                    e1.dma_start(out=dram_view(out_d, c, wl, wh), in_=hi)

            # software-pipelined schedule: chunk 1 runs half an iteration
            # behind chunk 0 so each chunk's ACT phase (Tanh / Ln Ln) overlaps
            # the other chunk's vector phase (products / updates).  The
            # wait-until timestamps steer the Tile list scheduler into that
            # stagger; they are lower bounds only, data deps still rule.
            S0, HALF, GAP = _SCHED
            for it in range(iters):
                for c in range(CHUNKS):
                    tA = S0 + (CHUNKS * it + c) * HALF
                    with tc.tile_wait_until(tA / 1e6, enable=tA > 0):
                        partA(c, it)
                    with tc.tile_wait_until((tA + GAP) / 1e6):
                        partB(c, it)

    _strip_syncs(nc)
    return nc


def _strip_syncs(nc):
    """walrus on this stack supports a single sync-wait slot per instruction.
    Reduce each instruction's wait list via a vector-clock pass: walking the
    scheduled program order, every engine accumulates knowledge of semaphore
    values - from its own queue position, from waits it has already performed,
    and transitively from the producer's knowledge snapshot at the awaited
    update.  A wait already implied by that knowledge is dropped.  Kernel-tail
    drains keep only their DMA wait (the per-engine drain + EVSEM butterfly
    that follows enforces engine completion)."""
    import bass_rust

    eng_sem = {"EngineType.DVE": "DVE_", "EngineType.Pool": "Pool_",
               "EngineType.Activation": "Activation_", "EngineType.PE": "PE_",
               "EngineType.SP": "SP_"}
    know = {e: {} for e in eng_sem}          # engine -> {sem: value}
    sem_hist = {}                            # sem -> list of (cum_value, snapshot)
    sem_cum = {}                             # sem -> cumulative inc so far

    # Sems that are ever decremented (barrier gather sems) are not monotone;
    # leave their waits untouched and keep them out of the knowledge model.
    nonmono = set()
    for b in nc.m.functions[0].blocks:
        for inst in b.instructions:
            si = inst.sync_info
            if si is not None:
                for u in si.on_update:
                    if u.update_mode != "sem-inc":
                        nonmono.add(u.ant_name)

    def implied(k, sem, val):
        return k.get(sem, 0) >= val

    def learn(k, sem, val):
        if k.get(sem, 0) < val:
            k[sem] = val
        # transitively absorb the producer's snapshot at this update
        hist = sem_hist.get(sem)
        if hist:
            import bisect
            i = bisect.bisect_left([h[0] for h in hist], val)
            if i < len(hist):
                for s2, v2 in hist[i][1].items():
                    if k.get(s2, 0) < v2:
                        k[s2] = v2

    from concourse import mybir

    for b in nc.m.functions[0].blocks:
        new_instructions = []
        for inst in b.instructions:
            si = inst.sync_info
            eng = str(inst.engine)
            k = know.setdefault(eng, {})
            if si is not None:
                waits = list(si.on_wait)
                if type(inst).__name__ == "InstDrain" and len(waits) > 1:
                    dma = [w for w in waits if "DMA" in w.ant_name]
                    keep_w = dma[-1:] if dma else waits[:1]
                    for w in waits:
                        learn(k, w.ant_name, w.wait_value)
                else:
                    merged = {}
                    for w in waits:
                        if w.ant_name in nonmono:
                            merged[id(w)] = w
                        elif w.ant_name not in merged or \
                                merged[w.ant_name].wait_value < w.wait_value:
                            merged[w.ant_name] = w
                    keep_w = []
                    for w in merged.values():
                        if w.ant_name in nonmono:
                            keep_w.append(w)
                            continue
                        if not implied(k, w.ant_name, w.wait_value):
                            keep_w.append(w)
                        learn(k, w.ant_name, w.wait_value)
                    # walrus has one wait slot per instruction: hoist extra
                    # waits onto injected no-ops on the same engine
                    while len(keep_w) > 1:
                        w = keep_w.pop(0)
                        nop = mybir.InstNoOp(
                            name=f"{inst.name}_w{len(keep_w)}",
                            engine=inst.engine, ins=[], outs=[],
                            sync_info=bass_rust.SyncInfo(
                                on_wait=[w], on_update=[]))
                        new_instructions.append(nop)
                if len(keep_w) != len(waits):
                    inst.sync_info = bass_rust.SyncInfo(
                        on_wait=keep_w, on_update=list(si.on_update))
                    si = inst.sync_info
                for u in si.on_update:
                    if u.update_mode == "sem-inc" and u.ant_name not in nonmono:
                        name = u.ant_name
                        cum = sem_cum.get(name, 0) + u.update_value
                        sem_cum[name] = cum
                        # own-engine sems are implicitly ordered for later
                        # instructions on the same queue
                        pref = eng_sem.get(eng)
                        if pref and name.startswith(pref):
                            k[name] = max(k.get(name, 0), cum)
                        sem_hist.setdefault(name, []).append((cum, dict(k)))
            new_instructions.append(inst)
        if len(new_instructions) != len(b.instructions):
            b.instructions = new_instructions


def kernel(llr, max_iters):
    llr = np.ascontiguousarray(np.asarray(llr), dtype=np.float32)
    iters = int(np.asarray(max_iters))
    B = llr.shape[0]
    if iters <= 0:
        return llr.reshape(B, 1, 7).copy()

    from concourse.bass_utils import run_bass_kernel_spmd

    Bc = B // NCORES
    key = (Bc, iters)
    if key not in _CACHE:
        _CACHE[key] = _build(Bc, iters)
    nc = _CACHE[key]

    flat = llr.reshape(B, 7)
    in_maps = [{"llr": flat[i * Bc:(i + 1) * Bc]} for i in range(NCORES)]
    res = run_bass_kernel_spmd(nc, in_maps, core_ids=list(range(NCORES)))
    out = np.concatenate([np.asarray(r["out"]) for r in res.results], axis=0)
    return out.reshape(B, 1, 7)


# revision 54
# speedup vs baseline: 1.0168x; 1.0009x over previous
"""LDPC belief-propagation kernel for Trainium2 (8 NeuronCores, data-parallel).

Math (per batch row, H fixed [3,7], 12 edges):
  t_e   = tanh(m_e / 2)                       (signed!)
  u_e   = prod_{e' in check c, e' != e} t_e'  (signed leave-one-out product)
  c2v_e = 2 atanh(u_e) = ln(1+u) - ln(1-u)    (signed, sign handled for free)
  new_llr_v = llr_v + sum_{c contains v} c2v_{c,v}
  m'_e  = new_llr_v - c2v_e
Only Tanh/Ln tables; the sign pipeline of the classic phi/phi formulation
disappears because the tanh products carry signs natively.

Edge layout is role-major per 12-slot group: [s0 s1 s2 | a0 a1 a2 | b0 b1 b2 |
d0 d1 d2] where s_c is the check's degree-1-variable edge (v0,v1,v3 - their
messages never change), d_c is v6's edge in check c, and (a, b) =
((v2c0, v2c1, v4c2), (v4c0, v5c1, v5c2)).  This makes every structural op a
single strided instruction:
  Q[k]   = T[k] * T[k+6]         (k=0..5: pair products (s*b, a*d) per check)
  U[3..8]  = T[(9,10,11,0,1,2)] * Q[0..5]   (loo for roles a, b)
  U[9..11] = T[3..5] * Q[0..2]              (loo for role d)
  M'[deg2 six edges] = LB6 + CV[partner]    (partner = pair-swap view)
  M'[d]  = (c2v_d-sum + llr_6) - CV[d]      (v6 leave-one-out via total sum)
Batch is split into 3 width-tuned chunks (87/93/76 columns per partition)
whose iterations run staggered, so ACT (Tanh + the two Ln ops), DVE
(products, c2v subtract, message updates) and Pool (d-role products, v6
pair-sums) stay concurrently busy; iteration 0 reads tanh(llr) directly so
the edge-slot scatter stays off the pipeline-fill critical path, and
setup copies / output DMAs are placed by schedule hints to keep the fill
and drain edges tight.  Sync info is reduced to walrus's one-wait-slot
limit by a vector-clock pass (_strip_syncs).
"""

import numpy as np

_CACHE = {}

NCORES = 8
P = 128      # partitions
CHUNKS = 3   # batch sub-chunks per core (pipeline depth)

# guard so ln(1 -+ 0.99999988*u) stays finite (>= ~1.2e-7) even at u = -+1
LNSCALE = 0.99999988

# (start, half-period, partA->partB gap) ns hints for the list scheduler
_SCHED = (0, 0, 0)

# schedule hints deferring chunk setup copies (ns)
_CPY = (0, 9800, 12300)

# manual chunk widths (must sum to Bc//P//1); None = near-even
_WS = (87, 93, 76)


def _build(Bc, iters):
    import contextlib

    import concourse.bass as bass
    import concourse.tile as tile
    from concourse import mybir
    from concourse.alu_op_type import AluOpType as Op

    F = mybir.ActivationFunctionType
    Wtot = Bc // P
    if _WS is not None and sum(_WS) == Wtot:
        Ws = list(_WS)
    else:
        base, rem = divmod(Wtot, CHUNKS)
        Ws = [base + (1 if i == 1 else 0) for i in range(CHUNKS)] \
            if rem == 1 else \
            [base + (1 if i >= CHUNKS - rem else 0) for i in range(CHUNKS)]
    f32 = mybir.dt.float32

    f16 = mybir.dt.float16
    nc = bass.Bass("TRN2", target_bir_lowering=False, debug=False,
                   num_devices=1)
    llr_d = nc.dram_tensor("llr", [Bc, 7], f32, kind="ExternalInput")
    out_d = nc.dram_tensor("out", [Bc, 7], f32, kind="ExternalOutput")

    def sub(t, off, dims):
        a = t[:] if callable(getattr(t, "__getitem__", None)) else t
        return bass.AP(tensor=a.tensor, offset=a.offset + off,
                       ap=[list(a.ap[0])] + [list(d) for d in dims])

    with tile.TileContext(nc) as tc:
        ctx = contextlib.ExitStack()
        with ctx:
            keep = ctx.enter_context(tc.tile_pool(name="keep", bufs=1))
            work = ctx.enter_context(tc.tile_pool(name="work", bufs=2))

            def K(name, c, k, dt=f32):
                return keep.tile([P, Ws[c] * k], dt, tag=name, name=name)

            # per-chunk persistent state
            LLs = [K(f"LL{c}", c, 7) for c in range(CHUNKS)]    # llr, natural v order
            LBs = [K(f"LB{c}", c, 6, f16) for c in range(CHUNKS)]   # llr bcast, deg2 edges
            L6s = [K(f"L6{c}", c, 3, f16) for c in range(CHUNKS)]   # llr6 bcast, v6 edges
            Ts  = [K(f"T{c}", c, 12) for c in range(CHUNKS)]    # tanh(m/2) per edge
            Ms  = [K(f"M{c}", c, 9, f16) for c in range(CHUNKS)]    # dyn messages
            NLs = [K(f"NL{c}", c, 7) for c in range(CHUNKS)]    # output llr

            act = nc.scalar.activation
            vec = nc.vector
            gps = nc.gpsimd

            def dram_view(t, c, w0, nw):
                # [P, nw*7] window of chunk c: rows base_c + p*Ws[c] + w
                a = t.ap()
                off = (P * sum(Ws[:c]) + w0) * 7
                return bass.AP(tensor=a.tensor, offset=a.offset + off,
                               ap=[[Ws[c] * 7, P], [1, nw * 7]])

            for c in range(CHUNKS):
                eng = nc.sync if c == 0 else nc.gpsimd
                eng.dma_start(out=LLs[c][:], in_=dram_view(llr_d, c, 0, Ws[c]))

            cur = {"W": Ws[0]}

            def v7(t, off, *dims):
                return sub(t, off, [[7, cur["W"]]] + [list(d) for d in dims])

            def v12(t, off, *dims):
                return sub(t, off, [[12, cur["W"]]] + [list(d) for d in dims])

            def v9(t, off, *dims):
                return sub(t, off, [[9, cur["W"]]] + [list(d) for d in dims])

            def v6(t, off, *dims):
                return sub(t, off, [[6, cur["W"]]] + [list(d) for d in dims])

            def v3(t, off, *dims):
                return sub(t, off, [[3, cur["W"]]] + [list(d) for d in dims])

            state = [{} for _ in range(CHUNKS)]

            def partA(c, it):
                """tanh + products: T, Q, U."""
                LL, LB, T, M = LLs[c], LBs[c], Ts[c], Ms[c]
                last = (it == iters - 1)
                W = Ws[c]
                cur["W"] = W
                Q = work.tile([P, W * 6], f32, tag=f"Q{c}", name="Q")
                U = work.tile([P, W * 12], f32, tag=f"U{c}", name="U")
                state[c] = {"Q": Q, "U": U}

                if it == 0:
                    # t = tanh(llr/2) once; iteration-0 products read TL
                    # directly so the T scatter stays off the critical path
                    TL = work.tile([P, W * 7], f32, tag=f"TL{c}", name="TL")
                    act(TL[:], LL[:], F.Tanh, scale=0.5)
                    vec.tensor_tensor(v6(Q, 0, [1, 2]), v7(TL, 0, [1, 2]),
                                      v7(TL, 4, [1, 2]), Op.mult)
                    vec.tensor_tensor(v6(Q, 2, [1, 1]), v7(TL, 3, [1, 1]),
                                      v7(TL, 5, [1, 1]), Op.mult)
                    gps.tensor_tensor(v6(Q, 3, [1, 2]), v7(TL, 2, [0, 2]),
                                      v7(TL, 6, [0, 2]), Op.mult)
                    gps.tensor_tensor(v6(Q, 5, [1, 1]), v7(TL, 4, [1, 1]),
                                      v7(TL, 6, [1, 1]), Op.mult)
                    vec.tensor_tensor(v12(U, 3, [1, 3]), v7(TL, 6, [0, 3]),
                                      v6(Q, 0, [1, 3]), Op.mult)
                    vec.tensor_tensor(v12(U, 6, [1, 2]), v7(TL, 0, [1, 2]),
                                      v6(Q, 3, [1, 2]), Op.mult)
                    vec.tensor_tensor(v12(U, 8, [1, 1]), v7(TL, 3, [1, 1]),
                                      v6(Q, 5, [1, 1]), Op.mult)
                    gps.tensor_tensor(v12(U, 9, [1, 2]), v7(TL, 2, [0, 2]),
                                      v6(Q, 0, [1, 2]), Op.mult)
                    gps.tensor_tensor(v12(U, 11, [1, 1]), v7(TL, 4, [1, 1]),
                                      v6(Q, 2, [1, 1]), Op.mult)
                    if last:  # iters == 1
                        vec.tensor_tensor(v12(U, 0, [1, 2]), v7(TL, 4, [1, 2]),
                                          v6(Q, 3, [1, 2]), Op.mult)
                        vec.tensor_tensor(v12(U, 2, [1, 1]), v7(TL, 5, [1, 1]),
                                          v6(Q, 5, [1, 1]), Op.mult)
                    # scatter t to role-major slots for later iterations,
                    # off the critical path (only statics strictly needed
                    # before iteration 1's products)
                    vec.tensor_copy(v12(T, 0, [1, 2]), v7(TL, 0, [1, 2]))
                    vec.tensor_copy(v12(T, 2, [1, 1]), v7(TL, 3, [1, 1]))
                    with tc.tile_wait_until(_CPY[c] / 1e6, enable=_CPY[c] > 0):
                        gps.tensor_copy(v6(LB, 0, [1, 2]), v7(LL, 2, [0, 2]))
                        gps.tensor_copy(v6(LB, 2, [1, 4]),
                                        v7(LL, 4, [1, 2], [0, 2]))
                        gps.tensor_copy(v3(L6s[c], 0, [1, 3]),
                                        v7(LL, 6, [0, 3]))
                else:
                    act(v12(T, 3, [1, 9]), M[:], F.Tanh, scale=0.5)
                    # pair products and signed leave-one-out products
                    vec.tensor_tensor(Q[:], v12(T, 0, [1, 6]),
                                      v12(T, 6, [1, 6]), Op.mult)
                    vec.tensor_tensor(v12(U, 3, [1, 6]),
                                      v12(T, 9, [-9, 2], [1, 3]),
                                      v6(Q, 0, [1, 6]), Op.mult)
                    gps.tensor_tensor(v12(U, 9, [1, 3]), v12(T, 3, [1, 3]),
                                      v6(Q, 0, [1, 3]), Op.mult)
                    if last:
                        vec.tensor_tensor(v12(U, 0, [1, 3]), v12(T, 6, [1, 3]),
                                          v6(Q, 3, [1, 3]), Op.mult)

            def partB(c, it):
                """c2v + message/new-llr update."""
                LL, LB, M, NL = LLs[c], LBs[c], Ms[c], NLs[c]
                last = (it == iters - 1)
                W = Ws[c]
                cur["W"] = W
                U = state[c]["U"]
                LP = work.tile([P, W * 12], f16, tag=f"LP{c}", name="LP")
                LM = work.tile([P, W * 12], f16, tag=f"LM{c}", name="LM")
                CV = work.tile([P, W * 12], f16, tag=f"CV{c}", name="CV")

                off, n = (0, 9) if last else (3, 6)
                # c2v = ln(1+u) - ln(1-u), guarded away from ln(0)
                act(v12(LP, off, [1, n + 3]), v12(U, off, [1, n + 3]), F.Ln,
                    bias=1.0, scale=LNSCALE)
                act(v12(LM, off, [1, n + 3]), v12(U, off, [1, n + 3]), F.Ln,
                    bias=1.0, scale=-LNSCALE)
                vec.tensor_tensor(v12(CV, off, [1, n + 3]),
                                  v12(LP, off, [1, n + 3]),
                                  v12(LM, off, [1, n + 3]), Op.subtract)

                # v6 leave-one-out sums of the d-role c2vs, depth 2:
                # X[0]=c10+c11  X[1]=c9+c11  X[2]=c9+c10
                X = work.tile([P, W * 3], f16, tag=f"X{c}", name="X")
                if not last:
                    # X[0], X[1] feed only the mid-iteration v6 m'-update
                    vec.tensor_tensor(v3(X, 0, [1, 2]), v12(CV, 10, [-1, 2]),
                                      v12(CV, 11, [0, 2]), Op.add)
                gps.tensor_tensor(v3(X, 2, [1, 1]), v12(CV, 9, [1, 1]),
                                  v12(CV, 10, [1, 1]), Op.add)

                if not last:
                    # m' for the six deg-2 edges: llr + partner c2v
                    vec.tensor_tensor(v9(M, 0, [1, 6]), v6(LB, 0, [1, 6]),
                                      v12(CV, 4, [2, 3], [-1, 2]), Op.add)
                    # m' for v6 edges: llr6 + sum of the other two c2v_d
                    vec.tensor_tensor(v9(M, 6, [1, 3]), v3(L6s[c], 0, [1, 3]),
                                      v3(X, 0, [1, 3]), Op.add)
                else:
                    # new_llr in natural variable order
                    SP = work.tile([P, W * 3], f32, tag=f"SP{c}", name="SP")
                    gps.tensor_tensor(v7(NL, 0, [1, 2]), v7(LL, 0, [1, 2]),
                                      v12(CV, 0, [1, 2]), Op.add)
                    gps.tensor_tensor(v7(NL, 3, [1, 1]), v7(LL, 3, [1, 1]),
                                      v12(CV, 2, [1, 1]), Op.add)
                    vec.tensor_tensor(SP[:], v12(CV, 3, [2, 3]),
                                      v12(CV, 4, [2, 3]), Op.add)
                    vec.tensor_tensor(v7(NL, 2, [1, 1]), v7(LL, 2, [1, 1]),
                                      v3(SP, 0, [1, 1]), Op.add)
                    vec.tensor_tensor(v7(NL, 4, [1, 2]), v7(LL, 4, [1, 2]),
                                      v3(SP, 1, [1, 2]), Op.add)
                    S1 = work.tile([P, W], f32, tag=f"S1{c}", name="S1")
                    vec.tensor_tensor(S1[:], v12(CV, 11, [1, 1]),
                                      v7(LL, 6, [1, 1]), Op.add)
                    vec.tensor_tensor(v7(NL, 6, [1, 1]), v3(X, 2, [1, 1]),
                                      S1[:], Op.add)
                    wl = W // 2
                    wh = W - wl
                    lo = bass.AP(tensor=NL[:].tensor, offset=NL[:].offset,
                                 ap=[list(NL[:].ap[0])] + [[7, wl], [1, 7]])
                    hi = bass.AP(tensor=NL[:].tensor,
                                 offset=NL[:].offset + wl * 7,
                                 ap=[list(NL[:].ap[0])] + [[7, wh], [1, 7]])
                    e0, e1 = ((nc.sync, nc.sync) if c < CHUNKS - 1
                              else (nc.sync, nc.gpsimd))
                    e0.dma_start(out=dram_view(out_d, c, 0, wl), in_=lo)
                    e1.dma_start(out=dram_view(out_d, c, wl, wh), in_=hi)

            # software-pipelined schedule: chunk 1 runs half an iteration
            # behind chunk 0 so each chunk's ACT phase (Tanh / Ln Ln) overlaps
            # the other chunk's vector phase (products / updates).  The
            # wait-until timestamps steer the Tile list scheduler into that
            # stagger; they are lower bounds only, data deps still rule.
            S0, HALF, GAP = _SCHED
            for it in range(iters):
                for c in range(CHUNKS):
                    tA = S0 + (CHUNKS * it + c) * HALF
                    with tc.tile_wait_until(tA / 1e6, enable=tA > 0):
                        partA(c, it)
                    with tc.tile_wait_until((tA + GAP) / 1e6):
                        partB(c, it)

    _strip_syncs(nc)
    return nc


def _strip_syncs(nc):
    """walrus on this stack supports a single sync-wait slot per instruction.
    Reduce each instruction's wait list via a vector-clock pass: walking the
    scheduled program order, every engine accumulates knowledge of semaphore
    values - from its own queue position, from waits it has already performed,
    and transitively from the producer's knowledge snapshot at the awaited
    update.  A wait already implied by that knowledge is dropped.  Kernel-tail
    drains keep only their DMA wait (the per-engine drain + EVSEM butterfly
    that follows enforces engine completion)."""
    import bass_rust

    eng_sem = {"EngineType.DVE": "DVE_", "EngineType.Pool": "Pool_",
               "EngineType.Activation": "Activation_", "EngineType.PE": "PE_",
               "EngineType.SP": "SP_"}
    know = {e: {} for e in eng_sem}          # engine -> {sem: value}
    sem_hist = {}                            # sem -> list of (cum_value, snapshot)
    sem_cum = {}                             # sem -> cumulative inc so far

    # Sems that are ever decremented (barrier gather sems) are not monotone;
    # leave their waits untouched and keep them out of the knowledge model.
    nonmono = set()
    for b in nc.m.functions[0].blocks:
        for inst in b.instructions:
            si = inst.sync_info
            if si is not None:
                for u in si.on_update:
                    if u.update_mode != "sem-inc":
                        nonmono.add(u.ant_name)

    def implied(k, sem, val):
        return k.get(sem, 0) >= val

    def learn(k, sem, val):
        if k.get(sem, 0) < val:
            k[sem] = val
        # transitively absorb the producer's snapshot at this update
        hist = sem_hist.get(sem)
        if hist:
            import bisect
            i = bisect.bisect_left([h[0] for h in hist], val)
            if i < len(hist):
                for s2, v2 in hist[i][1].items():
                    if k.get(s2, 0) < v2:
                        k[s2] = v2

    from concourse import mybir

    for b in nc.m.functions[0].blocks:
        new_instructions = []
        for inst in b.instructions:
            si = inst.sync_info
            eng = str(inst.engine)
            k = know.setdefault(eng, {})
            if si is not None:
                waits = list(si.on_wait)
                if type(inst).__name__ == "InstDrain" and len(waits) > 1:
                    dma = [w for w in waits if "DMA" in w.ant_name]
                    keep_w = dma[-1:] if dma else waits[:1]
                    for w in waits:
                        learn(k, w.ant_name, w.wait_value)
                else:
                    merged = {}
                    for w in waits:
                        if w.ant_name in nonmono:
                            merged[id(w)] = w
                        elif w.ant_name not in merged or \
                                merged[w.ant_name].wait_value < w.wait_value:
                            merged[w.ant_name] = w
                    keep_w = []
                    for w in merged.values():
                        if w.ant_name in nonmono:
                            keep_w.append(w)
                            continue
                        if not implied(k, w.ant_name, w.wait_value):
                            keep_w.append(w)
                        learn(k, w.ant_name, w.wait_value)
                    # walrus has one wait slot per instruction: hoist extra
                    # waits onto injected no-ops on the same engine
                    while len(keep_w) > 1:
                        w = keep_w.pop(0)
                        nop = mybir.InstNoOp(
                            name=f"{inst.name}_w{len(keep_w)}",
                            engine=inst.engine, ins=[], outs=[],
                            sync_info=bass_rust.SyncInfo(
                                on_wait=[w], on_update=[]))
                        new_instructions.append(nop)
                if len(keep_w) != len(waits):
                    inst.sync_info = bass_rust.SyncInfo(
                        on_wait=keep_w, on_update=list(si.on_update))
                    si = inst.sync_info
                for u in si.on_update:
                    if u.update_mode == "sem-inc" and u.ant_name not in nonmono:
                        name = u.ant_name
                        cum = sem_cum.get(name, 0) + u.update_value
                        sem_cum[name] = cum
                        # own-engine sems are implicitly ordered for later
                        # instructions on the same queue
                        pref = eng_sem.get(eng)
                        if pref and name.startswith(pref):
                            k[name] = max(k.get(name, 0), cum)
                        sem_hist.setdefault(name, []).append((cum, dict(k)))
            new_instructions.append(inst)
        if len(new_instructions) != len(b.instructions):
            b.instructions = new_instructions


def kernel(llr, max_iters):
    llr = np.ascontiguousarray(np.asarray(llr), dtype=np.float32)
    iters = int(np.asarray(max_iters))
    B = llr.shape[0]
    if iters <= 0:
        return llr.reshape(B, 1, 7).copy()

    from concourse.bass_utils import run_bass_kernel_spmd

    Bc = B // NCORES
    key = (Bc, iters)
    if key not in _CACHE:
        _CACHE[key] = _build(Bc, iters)
    nc = _CACHE[key]

    flat = llr.reshape(B, 7)
    in_maps = [{"llr": flat[i * Bc:(i + 1) * Bc]} for i in range(NCORES)]
    res = run_bass_kernel_spmd(nc, in_maps, core_ids=list(range(NCORES)))
    out = np.concatenate([np.asarray(r["out"]) for r in res.results], axis=0)
    return out.reshape(B, 1, 7)
